# revision 14
# baseline (speedup 1.0000x reference)
"""Causal self-attention layer (B=4, T=2048, C=1024, H=16) on 8 TRN2 NeuronCores.

Sharding: Megatron-style tensor parallel over heads — 2 heads per core.
Each core computes q/k/v projections for its 2 heads, causal flash-style
attention with ones-columns on V to accumulate softmax denominators, and a
partial output projection against its 128-row slice of W_proj. The host sums
the 8 partial projections and adds b_proj.

All matmul operands are bfloat16 (pre-cast on host for x/weights; on-chip
activations write bf16 directly out of PSUM). fp32r matmuls run in
fp32_mode=HIGH which power-throttles the PE to 50% for most of the kernel;
bf16 keeps the PE mostly at full rate, halves the x/out DMA bytes, and
doubles DVE throughput on the mask multiplies.

Schedule: the q/k/v projection tiles for batch b+1 are interleaved into the
attention stream of batch b, so the PE never runs a long unbroken stream of
dense projection matmuls (which exhausts the power budget and triggers the
50%-duty throttle), and no phase boundary stalls the PE. V tiles are
transposed once per batch ([128,128] both-heads transpose) with two ones
columns (at free offsets 64 and 129) so each head's 65-wide PV lhsT slice
puts y in psum rows 0-63 and the softmax denominator in row 64. The V
transposes for batch b+1 are emitted in the denominator-reciprocal latency
gap of batch b. PSUM: 4 pools x 2 bufs x 2KB = exactly 8 banks.
"""
import sys

sys.path.insert(0, "/opt/trn_rl_repo")

import numpy as np
import ml_dtypes

import concourse.bass as bass  # noqa: F401
from concourse import bacc
import concourse.mybir as mybir
import concourse.tile as tile
from concourse.bass_utils import run_bass_kernel_spmd
from concourse.masks import make_identity

B, T, C = 4, 2048, 1024
H, DH = 16, 64
N_CORES = 8
HPC = H // N_CORES          # heads per core = 2
DPC = HPC * DH              # head-dims per core = 128
NT = B * T                  # 8192 tokens
CH = C // 128               # 8 contraction chunks
QB = 512                    # q-block width (moving dim)
KT = 128                    # k-tile width (PE partition dim)
CHUNK = 8                   # k-tiles per S/PV emission chunk
SCALE = 1.0 / 8.0           # 1/sqrt(DH)
TPB = T // QB               # qkv token tiles per batch = 4

F32 = mybir.dt.float32
BF16 = mybir.dt.bfloat16
AF = mybir.ActivationFunctionType
BF16_NP = ml_dtypes.bfloat16

_CACHED_NC = None
LAST_RESULT = None


def _build():
    nc = bacc.Bacc(None)

    xT = nc.dram_tensor("xT", [C, NT], BF16, kind="ExternalInput")
    # qkv weights pre-arranged on host to the SBUF layout [p, c, n]
    wq = nc.dram_tensor("wq", [128, CH, DPC], BF16, kind="ExternalInput")
    wk = nc.dram_tensor("wk", [128, CH, DPC], BF16, kind="ExternalInput")
    wv = nc.dram_tensor("wv", [128, CH, DPC], BF16, kind="ExternalInput")
    bq = nc.dram_tensor("bq", [DPC, 1], F32, kind="ExternalInput")
    bk = nc.dram_tensor("bk", [DPC, 1], F32, kind="ExternalInput")
    bv = nc.dram_tensor("bv", [DPC, 1], F32, kind="ExternalInput")
    wp = nc.dram_tensor("wp", [DPC, C], BF16, kind="ExternalInput")
    emat_in = nc.dram_tensor("emat", [8, 4, 128], BF16, kind="ExternalInput")
    out = nc.dram_tensor("out", [NT, C], BF16, kind="ExternalOutput")

    with tile.TileContext(nc) as tc:
        with (
            tc.tile_pool(name="const", bufs=1) as const,
            tc.tile_pool(name="res", bufs=1) as res,
        ):
            # --- constants (built in f32, cast to bf16 once) ---
            ident = const.tile([128, 128], BF16, tag="ident")
            # sliding causal mask: wmask[k, u] = 1 iff k <= u - 512; a crossing
            # tile r multiplies by wmask[:, 512-128r : 1024-128r]
            wmask = const.tile([128, 1024], BF16, tag="wmask")
            ones_col = const.tile([128, 1], BF16, tag="ones_col")
            # indicator lhsT per q-block: Emat[:, qb, j] selects den row qb
            # (head 0) for j<64 and row 4+qb (head 1) for j>=64, so one matmul
            # broadcasts both heads' reciprocals into a [128, 512] tile
            emat = const.tile([8, 4, 128], BF16, tag="emat")
            with tc.tile_pool(name="cstage", bufs=1) as cstage:
                ident_s = cstage.tile([128, 128], F32, tag="ident_s")
                make_identity(nc, ident_s[:])
                nc.vector.tensor_copy(ident[:], ident_s[:])

                wmask_s = cstage.tile([128, 1024], F32, tag="wmask_s")
                nc.gpsimd.memset(wmask_s[:], 0.0)
                nc.gpsimd.affine_select(
                    out=wmask_s[:],
                    in_=wmask_s[:],
                    compare_op=mybir.AluOpType.is_gt,
                    fill=1.0,
                    base=512,
                    # keep 0 where (512 + k - u) > 0, fill 1 where k <= u - 512
                    pattern=[[-1, 1024]],
                    channel_multiplier=1,
                )
                nc.vector.tensor_copy(wmask[:], wmask_s[:])

                ones_s = cstage.tile([128, 1], F32, tag="ones_s")
                nc.gpsimd.memset(ones_s[:], 1.0)
                nc.vector.tensor_copy(ones_col[:], ones_s[:])

            bq_t = const.tile([DPC, 1], F32, tag="bq")
            bk_t = const.tile([DPC, 1], F32, tag="bk")
            bv_t = const.tile([DPC, 1], F32, tag="bv")
            nc.sync.dma_start(bq_t[:], bq[:])
            nc.sync.dma_start(bk_t[:], bk[:])
            nc.sync.dma_start(bv_t[:], bv[:])

            # weights -> SBUF directly in bf16 (cast on host)
            wq_r = const.tile([128, CH, DPC], BF16, tag="wq_r")
            wk_r = const.tile([128, CH, DPC], BF16, tag="wk_r")
            wv_r = const.tile([128, CH, DPC], BF16, tag="wv_r")
            wp_r = const.tile([DPC, C], BF16, tag="wp_r")
            for w_in, w_dst in ((wq, wq_r), (wk, wk_r), (wv, wv_r)):
                nc.sync.dma_start(w_dst[:], w_in[:])
            nc.sync.dma_start(wp_r[:], wp[:])
            nc.sync.dma_start(emat[:], emat_in[:])

            # --- residents ---
            qT = res.tile([DPC, NT], BF16, tag="qT")
            kT = res.tile([DPC, NT], BF16, tag="kT")
            vT = res.tile([DPC, NT], BF16, tag="vT")
            yT = res.tile([DPC, NT], BF16, tag="yT")

            xT_re = xT.rearrange("(c p) t -> p c t", p=128)
            n_ktiles = T // KT  # 16

            with (
                tc.tile_pool(name="xpool", bufs=3) as xpool,
                tc.tile_pool(name="vpool", bufs=34) as vpool,
                tc.tile_pool(name="epool", bufs=CHUNK + 3) as epool,
                tc.tile_pool(name="dpool", bufs=2) as dpool,
                tc.tile_pool(name="opool", bufs=6) as opool,
                tc.tile_pool(name="q_psum", bufs=2, space="PSUM") as q_psum,
                tc.tile_pool(name="s_psum", bufs=3, space="PSUM") as s_psum,
                tc.tile_pool(name="y_psum", bufs=1, space="PSUM") as y_psum,
                tc.tile_pool(name="p_psum", bufs=2, space="PSUM") as p_psum,
            ):
                xs_tiles = {}

                def dma_x(tt):
                    if tt >= NT // QB or tt in xs_tiles:
                        return
                    xs = xpool.tile([128, CH, QB], BF16, tag="xs", name=f"xs{tt}")
                    nc.sync.dma_start(
                        xs[:], xT_re[:, :, tt * QB : (tt + 1) * QB]
                    )
                    xs_tiles[tt] = xs

                def qkv_tile(tt):
                    """Project one 512-token tile into qT/kT/vT; prefetch x."""
                    dma_x(tt + 1)
                    xs = xs_tiles.pop(tt)
                    ts_ = slice(tt * QB, (tt + 1) * QB)
                    psq = q_psum.tile([128, QB], F32, tag="qkv", name=f"psq{tt}")
                    psk = q_psum.tile([128, QB], F32, tag="qkv", name=f"psk{tt}")
                    psv = q_psum.tile([128, QB], F32, tag="qkv", name=f"psv{tt}")
                    for ps, w_r in ((psq, wq_r), (psk, wk_r), (psv, wv_r)):
                        for c in range(CH):
                            nc.tensor.matmul(
                                ps[:], w_r[:, c, :], xs[:, c, :],
                                start=(c == 0), stop=(c == CH - 1),
                            )
                    # copy out of PSUM (+bias; q also scaled by 1/sqrt(dh))
                    nc.scalar.activation(qT[:, ts_], psq[:], AF.Identity, bias=bq_t[:], scale=SCALE)
                    nc.vector.tensor_scalar_add(kT[:, ts_], psk[:], bk_t[:])
                    nc.vector.tensor_scalar_add(vT[:, ts_], psv[:], bv_t[:])

                # per-batch state
                vts_all = {}   # b -> list of 16 [128, 130] tiles
                den_all = {}   # b -> (denw, den)

                def vts_half(b, half):
                    """Transpose 8 V token-tiles (both heads at once).

                    v tile layout [128 tok, 130]: cols 0-63 head0 dims, col 64
                    ones, cols 65-128 head1 dims, col 129 ones. Head hl's PV
                    lhsT is v[:, 65*hl : 65*hl+65] -> psum rows 0-63 = y,
                    row 64 = denominator.
                    """
                    cb = b * T
                    vts = vts_all.setdefault(b, [None] * n_ktiles)
                    for kt in range(half * 8, half * 8 + 8):
                        pt = s_psum.tile([128, 512], BF16, tag="s", name=f"pt{b}_{kt}")
                        nc.tensor.transpose(
                            pt[:, :128],
                            vT[:, cb + kt * KT : cb + (kt + 1) * KT],
                            ident[:],
                        )
                        v = vpool.tile([128, 130], BF16, tag="v", name=f"v{b}_{kt}")
                        nc.vector.tensor_copy(v[:, 0:64], pt[:, 0:64])
                        nc.scalar.copy(v[:, 65:129], pt[:, 64:128])
                        nc.vector.tensor_copy(v[:, 64:65], ones_col[:])
                        nc.vector.tensor_copy(v[:, 129:130], ones_col[:])
                        vts[kt] = v

                def sp_unit(b, hl, qb):
                    """Scores + exp + P@V for one (head, q-block)."""
                    cb = b * T
                    rb = hl * DH
                    vts = vts_all[b]
                    qs = slice(cb + qb * QB, cb + (qb + 1) * QB)
                    py = y_psum.tile([128, QB], F32, tag="py", name=f"py{b}_{hl}_{qb}")
                    nkt = (qb + 1) * (QB // KT)
                    for k0 in range(0, nkt, CHUNK):
                        kts = range(k0, min(k0 + CHUNK, nkt))
                        exs = {}
                        # scores + exp for this chunk
                        for kt in kts:
                            ps = s_psum.tile([128, QB], F32, tag="s", name=f"ps{kt}")
                            nc.tensor.matmul(
                                ps[:],
                                kT[rb : rb + DH, cb + kt * KT : cb + (kt + 1) * KT],
                                qT[rb : rb + DH, qs],
                                start=True,
                                stop=True,
                            )
                            ex = epool.tile([128, QB], BF16, tag="ex", name=f"ex{kt}")
                            nc.scalar.activation(ex[:], ps[:], AF.Exp)
                            r = kt - qb * (QB // KT)
                            if r >= 0:
                                # diagonal-crossing tile: zero out k > q
                                nc.vector.tensor_mul(
                                    ex[:], ex[:],
                                    wmask[:, 512 - r * KT : 1024 - r * KT],
                                )
                            exs[kt] = ex
                        # grouped P@V accumulation for this chunk
                        for kt in kts:
                            nc.tensor.matmul(
                                py[: DH + 1],
                                vts[kt][:, 65 * hl : 65 * hl + 65],
                                exs[kt][:],
                                start=(kt == 0),
                                stop=(kt == nkt - 1),
                            )
                    # stash unnormalized y; scatter the denominator row to its
                    # final partition right away (per-unit, off critical path)
                    denw, den = den_all[b]
                    p = hl * 4 + qb
                    nc.vector.tensor_copy(
                        denw[:, p * QB : (p + 1) * QB], py[DH : DH + 1, :]
                    )
                    nc.vector.tensor_copy(yT[rb : rb + DH, qs], py[:DH, :])
                    nc.sync.dma_start(
                        den[p : p + 1, :], denw[:, p * QB : (p + 1) * QB]
                    )

                def den_prep(b):
                    """Reciprocal of the scattered denominators (fast approx)."""
                    _denw, den = den_all[b]
                    recf = dpool.tile([8, QB], F32, tag="recf", name=f"recf{b}")
                    nc.vector.reciprocal_approx_fast(recf[:], den[:])
                    rec = dpool.tile([8, QB], BF16, tag="rec", name=f"rec{b}")
                    nc.scalar.copy(rec[:], recf[:])
                    return rec

                def norm(b, rec):
                    cb = b * T
                    for qb in range(T // QB):
                        qs = slice(cb + qb * QB, cb + (qb + 1) * QB)
                        pb = p_psum.tile([128, 512], F32, tag="p", name=f"pb{b}_{qb}")
                        nc.tensor.matmul(
                            pb[:, :QB], emat[:, qb, :], rec[:],
                            start=True, stop=True,
                        )
                        nc.vector.tensor_mul(yT[:, qs], yT[:, qs], pb[:, :QB])

                def proj_quarter(b, i):
                    """Output projection for 4 of the batch's 16 token tiles."""
                    cb = b * T
                    for tt in range(i * 4, i * 4 + 4):
                        trow = cb + tt * 128
                        for half in range(2):
                            pp = p_psum.tile([128, 512], F32, tag="p", name=f"pp{b}_{tt}_{half}")
                            nc.tensor.matmul(
                                pp[:],
                                yT[:, trow : trow + 128],
                                wp_r[:, half * 512 : (half + 1) * 512],
                                start=True,
                                stop=True,
                            )
                            os_ = opool.tile([128, 512], BF16, tag="os", name=f"os{b}_{tt}_{half}")
                            nc.vector.tensor_copy(os_[:], pp[:])
                            nc.sync.dma_start(
                                out[trow : trow + 128, half * 512 : (half + 1) * 512],
                                os_[:],
                            )

                # ================= emission schedule =================
                dma_x(0)
                for tt in range(TPB):          # qkv for batch 0
                    qkv_tile(tt)
                for b in range(B):
                    denw = dpool.tile([1, 8 * QB], F32, tag="denw", name=f"denw{b}")
                    den = dpool.tile([8, QB], F32, tag="den", name=f"den{b}")
                    den_all[b] = (denw, den)
                    if b == 0:
                        vts_half(0, 0)
                        vts_half(0, 1)
                    # interleave next batch's qkv tiles into this batch's
                    # attention stream (power smoothing + overlap); qb-ascending
                    # order so the next batch's vts halves unblock sps in order
                    sps = [(hl, qb) for qb in range(T // QB) for hl in range(HPC)]
                    qnext = list(range((b + 1) * TPB, (b + 2) * TPB)) if b + 1 < B else []
                    qpos = {0: 1, 1: 3, 2: 5, 3: 6}  # after sp index i emit qkv tile
                    for i, (hl, qb) in enumerate(sps):
                        sp_unit(b, hl, qb)
                        for j, pos in qpos.items():
                            if pos == i and j < len(qnext):
                                qkv_tile(qnext[j])
                    rec = den_prep(b)
                    norm(b, rec)
                    # next batch's V transposes slot between proj quarters
                    proj_quarter(b, 0)
                    if b + 1 < B:
                        vts_half(b + 1, 0)
                    proj_quarter(b, 1)
                    if b + 1 < B:
                        vts_half(b + 1, 1)
                        vts_all.pop(b, None)
                    proj_quarter(b, 2)
                    proj_quarter(b, 3)

    nc.compile()
    return nc


def _get_nc():
    global _CACHED_NC
    if _CACHED_NC is None:
        _CACHED_NC = _build()
    return _CACHED_NC


def kernel(x, W_qkv, b_qkv, W_proj, b_proj, _trace=False, _core_ids=None):
    global LAST_RESULT
    x = np.asarray(x, dtype=np.float32)
    W_qkv = np.asarray(W_qkv, dtype=np.float32)
    b_qkv = np.asarray(b_qkv, dtype=np.float32)
    W_proj = np.asarray(W_proj, dtype=np.float32)
    b_proj = np.asarray(b_proj, dtype=np.float32)

    xT = np.ascontiguousarray(x.reshape(NT, C).T).astype(BF16_NP)
    W_qkv_b = W_qkv.astype(BF16_NP)
    W_proj_b = W_proj.astype(BF16_NP)
    emat_np = np.zeros((8, 4, 128), dtype=BF16_NP)
    for qb in range(4):
        emat_np[qb, qb, :DH] = 1.0
        emat_np[4 + qb, qb, DH:] = 1.0
    core_ids = list(range(N_CORES)) if _core_ids is None else _core_ids

    def w_pcn(col0, col1):
        # [C, DPC] -> SBUF layout [p=128, c=CH, n=DPC], contiguous
        w = W_qkv_b[:, col0:col1].reshape(CH, 128, DPC)
        return np.ascontiguousarray(w.transpose(1, 0, 2))

    in_maps = []
    for core in range(len(core_ids)):
        s = slice(core * DPC, (core + 1) * DPC)
        in_maps.append(
            {
                "xT": xT,
                "wq": w_pcn(0 * C + core * DPC, 0 * C + (core + 1) * DPC),
                "wk": w_pcn(1 * C + core * DPC, 1 * C + (core + 1) * DPC),
                "wv": w_pcn(2 * C + core * DPC, 2 * C + (core + 1) * DPC),
                # device computes qT = psq*SCALE + bias, so pre-scale the q bias
                "bq": np.ascontiguousarray(b_qkv[0 * C + core * DPC : 0 * C + (core + 1) * DPC, None]) * np.float32(SCALE),
                "bk": np.ascontiguousarray(b_qkv[1 * C + core * DPC : 1 * C + (core + 1) * DPC, None]),
                "bv": np.ascontiguousarray(b_qkv[2 * C + core * DPC : 2 * C + (core + 1) * DPC, None]),
                "wp": np.ascontiguousarray(W_proj_b[s, :]),
                "emat": emat_np,
            }
        )

    nc = _get_nc()
    res = run_bass_kernel_spmd(nc, in_maps, core_ids, trace=_trace)
    LAST_RESULT = res

    acc = np.zeros((NT, C), dtype=np.float64)
    for r in res.results:
        acc += r["out"].astype(np.float64)
    acc += b_proj.astype(np.float64)
    return acc.reshape(B, T, C).astype(np.float32)


# revision 17
# speedup vs baseline: 1.0709x; 1.0709x over previous
"""Causal self-attention layer (B=4, T=2048, C=1024, H=16) on 8 TRN2 NeuronCores.

Sharding: Megatron-style tensor parallel over heads — 2 heads per core.
Each core computes q/k/v projections for its 2 heads, causal flash-style
attention with ones-columns on V to accumulate softmax denominators, and a
partial output projection against its 128-row slice of W_proj. The host sums
the 8 partial projections and adds b_proj.

All matmul operands are bfloat16 (pre-cast on host for x/weights; on-chip
activations write bf16 directly out of PSUM). fp32r matmuls run in
fp32_mode=HIGH which power-throttles the PE to 50% for most of the kernel;
bf16 keeps the PE mostly at full rate, halves the x/out DMA bytes, and
doubles DVE throughput on the mask multiplies.

Schedule: the q/k/v projection tiles for batch b+1 are interleaved into the
attention stream of batch b, so the PE never runs a long unbroken stream of
dense projection matmuls (which exhausts the power budget and triggers the
50%-duty throttle), and no phase boundary stalls the PE. V tiles are
transposed once per batch ([128,128] both-heads transpose) with two ones
columns (at free offsets 64 and 129) so each head's 65-wide PV lhsT slice
puts y in psum rows 0-63 and the softmax denominator in row 64. The V
transposes for batch b+1 are emitted in the denominator-reciprocal latency
gap of batch b. PSUM: 4 pools x 2 bufs x 2KB = exactly 8 banks.
"""
import sys

sys.path.insert(0, "/opt/trn_rl_repo")

import numpy as np
import ml_dtypes

import concourse.bass as bass  # noqa: F401
from concourse import bacc
import concourse.mybir as mybir
import concourse.tile as tile
from concourse.bass_utils import run_bass_kernel_spmd
from concourse.masks import make_identity

B, T, C = 4, 2048, 1024
H, DH = 16, 64
N_CORES = 8
HPC = H // N_CORES          # heads per core = 2
DPC = HPC * DH              # head-dims per core = 128
NT = B * T                  # 8192 tokens
CH = C // 128               # 8 contraction chunks
QB = 512                    # q-block width (moving dim)
KT = 128                    # k-tile width (PE partition dim)
CHUNK = 8                   # k-tiles per S/PV emission chunk
SCALE = 1.0 / 8.0           # 1/sqrt(DH)
TPB = T // QB               # qkv token tiles per batch = 4

F32 = mybir.dt.float32
BF16 = mybir.dt.bfloat16
AF = mybir.ActivationFunctionType
BF16_NP = ml_dtypes.bfloat16

_CACHED_NC = None
LAST_RESULT = None


def _build():
    nc = bacc.Bacc(None)

    xT = nc.dram_tensor("xT", [C, NT], BF16, kind="ExternalInput")
    # qkv weights pre-arranged on host to the SBUF layout [p, c, n]
    wq = nc.dram_tensor("wq", [128, CH, DPC], BF16, kind="ExternalInput")
    wk = nc.dram_tensor("wk", [128, CH, DPC], BF16, kind="ExternalInput")
    wv = nc.dram_tensor("wv", [128, CH, DPC], BF16, kind="ExternalInput")
    bq = nc.dram_tensor("bq", [DPC, 1], F32, kind="ExternalInput")
    bk = nc.dram_tensor("bk", [DPC, 1], F32, kind="ExternalInput")
    bv = nc.dram_tensor("bv", [DPC, 1], F32, kind="ExternalInput")
    wp = nc.dram_tensor("wp", [DPC, C], BF16, kind="ExternalInput")
    emat_in = nc.dram_tensor("emat", [8, 4, 128], BF16, kind="ExternalInput")
    out = nc.dram_tensor("out", [NT, C], BF16, kind="ExternalOutput")

    with tile.TileContext(nc) as tc:
        with (
            tc.tile_pool(name="const", bufs=1) as const,
            tc.tile_pool(name="res", bufs=1) as res,
        ):
            # --- constants (built in f32, cast to bf16 once) ---
            ident = const.tile([128, 128], BF16, tag="ident")
            # sliding causal mask: wmask[k, u] = 1 iff k <= u - 512; a crossing
            # tile r multiplies by wmask[:, 512-128r : 1024-128r]
            wmask = const.tile([128, 1024], BF16, tag="wmask")
            ones_col = const.tile([128, 1], BF16, tag="ones_col")
            # indicator lhsT per q-block: Emat[:, qb, j] selects den row qb
            # (head 0) for j<64 and row 4+qb (head 1) for j>=64, so one matmul
            # broadcasts both heads' reciprocals into a [128, 512] tile
            emat = const.tile([8, 4, 128], BF16, tag="emat")
            with tc.tile_pool(name="cstage", bufs=1) as cstage:
                ident_s = cstage.tile([128, 128], F32, tag="ident_s")
                make_identity(nc, ident_s[:])
                nc.vector.tensor_copy(ident[:], ident_s[:])

                wmask_s = cstage.tile([128, 1024], F32, tag="wmask_s")
                nc.gpsimd.memset(wmask_s[:], 0.0)
                nc.gpsimd.affine_select(
                    out=wmask_s[:],
                    in_=wmask_s[:],
                    compare_op=mybir.AluOpType.is_gt,
                    fill=1.0,
                    base=512,
                    # keep 0 where (512 + k - u) > 0, fill 1 where k <= u - 512
                    pattern=[[-1, 1024]],
                    channel_multiplier=1,
                )
                nc.vector.tensor_copy(wmask[:], wmask_s[:])

                ones_s = cstage.tile([128, 1], F32, tag="ones_s")
                nc.gpsimd.memset(ones_s[:], 1.0)
                nc.vector.tensor_copy(ones_col[:], ones_s[:])

            bq_t = const.tile([DPC, 1], F32, tag="bq")
            bk_t = const.tile([DPC, 1], F32, tag="bk")
            bv_t = const.tile([DPC, 1], F32, tag="bv")
            nc.sync.dma_start(bq_t[:], bq[:])
            nc.sync.dma_start(bk_t[:], bk[:])
            nc.sync.dma_start(bv_t[:], bv[:])

            # weights -> SBUF directly in bf16 (cast on host)
            wq_r = const.tile([128, CH, DPC], BF16, tag="wq_r")
            wk_r = const.tile([128, CH, DPC], BF16, tag="wk_r")
            wv_r = const.tile([128, CH, DPC], BF16, tag="wv_r")
            wp_r = const.tile([DPC, C], BF16, tag="wp_r")
            for w_in, w_dst in ((wq, wq_r), (wk, wk_r), (wv, wv_r)):
                nc.sync.dma_start(w_dst[:], w_in[:])
            nc.sync.dma_start(wp_r[:], wp[:])
            nc.sync.dma_start(emat[:], emat_in[:])

            # --- residents ---
            qT = res.tile([DPC, NT], BF16, tag="qT")
            kT = res.tile([DPC, NT], BF16, tag="kT")
            vT = res.tile([DPC, NT], BF16, tag="vT")
            yT = res.tile([DPC, NT], BF16, tag="yT")

            xT_re = xT.rearrange("(c p) t -> p c t", p=128)
            n_ktiles = T // KT  # 16

            with (
                tc.tile_pool(name="xpool", bufs=3) as xpool,
                tc.tile_pool(name="vpool", bufs=34) as vpool,
                tc.tile_pool(name="epool", bufs=CHUNK + 3) as epool,
                tc.tile_pool(name="dpool", bufs=2) as dpool,
                tc.tile_pool(name="opool", bufs=6) as opool,
                tc.tile_pool(name="q_psum", bufs=2, space="PSUM") as q_psum,
                tc.tile_pool(name="s_psum", bufs=3, space="PSUM") as s_psum,
                tc.tile_pool(name="y_psum", bufs=1, space="PSUM") as y_psum,
                tc.tile_pool(name="p_psum", bufs=2, space="PSUM") as p_psum,
            ):
                xs_tiles = {}

                def dma_x(tt):
                    if tt >= NT // QB or tt in xs_tiles:
                        return
                    xs = xpool.tile([128, CH, QB], BF16, tag="xs", name=f"xs{tt}")
                    nc.sync.dma_start(
                        xs[:], xT_re[:, :, tt * QB : (tt + 1) * QB]
                    )
                    xs_tiles[tt] = xs

                def qkv_tile(tt):
                    """Project one 512-token tile into qT/kT/vT; prefetch x."""
                    dma_x(tt + 1)
                    xs = xs_tiles.pop(tt)
                    ts_ = slice(tt * QB, (tt + 1) * QB)
                    psq = q_psum.tile([128, QB], F32, tag="qkv", name=f"psq{tt}")
                    psk = q_psum.tile([128, QB], F32, tag="qkv", name=f"psk{tt}")
                    psv = q_psum.tile([128, QB], F32, tag="qkv", name=f"psv{tt}")
                    for ps, w_r in ((psq, wq_r), (psk, wk_r), (psv, wv_r)):
                        for c in range(CH):
                            nc.tensor.matmul(
                                ps[:], w_r[:, c, :], xs[:, c, :],
                                start=(c == 0), stop=(c == CH - 1),
                            )
                    # copy out of PSUM (+bias; q also scaled by 1/sqrt(dh))
                    nc.scalar.activation(qT[:, ts_], psq[:], AF.Identity, bias=bq_t[:], scale=SCALE)
                    nc.vector.tensor_scalar_add(kT[:, ts_], psk[:], bk_t[:])
                    nc.vector.tensor_scalar_add(vT[:, ts_], psv[:], bv_t[:])

                # per-batch state
                vts_all = {}   # b -> list of 16 [128, 130] tiles
                den_all = {}   # b -> (denw, den)

                def vts_half(b, half):
                    """Transpose 8 V token-tiles (both heads at once).

                    v tile layout [128 tok, 130]: cols 0-63 head0 dims, col 64
                    ones, cols 65-128 head1 dims, col 129 ones. Head hl's PV
                    lhsT is v[:, 65*hl : 65*hl+65] -> psum rows 0-63 = y,
                    row 64 = denominator.
                    """
                    cb = b * T
                    vts = vts_all.setdefault(b, [None] * n_ktiles)
                    for kt in range(half * 8, half * 8 + 8):
                        pt = s_psum.tile([128, 512], BF16, tag="s", name=f"pt{b}_{kt}")
                        nc.tensor.transpose(
                            pt[:, :128],
                            vT[:, cb + kt * KT : cb + (kt + 1) * KT],
                            ident[:],
                        )
                        v = vpool.tile([128, 130], BF16, tag="v", name=f"v{b}_{kt}")
                        nc.vector.tensor_copy(v[:, 0:64], pt[:, 0:64])
                        nc.scalar.copy(v[:, 65:129], pt[:, 64:128])
                        nc.vector.tensor_copy(v[:, 64:65], ones_col[:])
                        nc.vector.tensor_copy(v[:, 129:130], ones_col[:])
                        vts[kt] = v

                def sp_unit(b, hl, qb):
                    """Scores + exp + P@V for one (head, q-block)."""
                    cb = b * T
                    rb = hl * DH
                    vts = vts_all[b]
                    qs = slice(cb + qb * QB, cb + (qb + 1) * QB)
                    py = y_psum.tile([128, QB], F32, tag="py", name=f"py{b}_{hl}_{qb}")
                    nkt = (qb + 1) * (QB // KT)
                    for k0 in range(0, nkt, CHUNK):
                        kts = range(k0, min(k0 + CHUNK, nkt))
                        exs = {}
                        # scores + exp for this chunk
                        for kt in kts:
                            ps = s_psum.tile([128, QB], F32, tag="s", name=f"ps{kt}")
                            nc.tensor.matmul(
                                ps[:],
                                kT[rb : rb + DH, cb + kt * KT : cb + (kt + 1) * KT],
                                qT[rb : rb + DH, qs],
                                start=True,
                                stop=True,
                            )
                            ex = epool.tile([128, QB], BF16, tag="ex", name=f"ex{kt}")
                            nc.scalar.activation(ex[:], ps[:], AF.Exp)
                            r = kt - qb * (QB // KT)
                            if r >= 0:
                                # diagonal-crossing tile: zero out k > q
                                nc.vector.tensor_mul(
                                    ex[:], ex[:],
                                    wmask[:, 512 - r * KT : 1024 - r * KT],
                                )
                            exs[kt] = ex
                        # grouped P@V accumulation for this chunk
                        for kt in kts:
                            nc.tensor.matmul(
                                py[: DH + 1],
                                vts[kt][:, 65 * hl : 65 * hl + 65],
                                exs[kt][:],
                                start=(kt == 0),
                                stop=(kt == nkt - 1),
                            )
                    # stash unnormalized y; scatter the denominator row to its
                    # final partition right away (per-unit, off critical path)
                    denw, den = den_all[b]
                    p = hl * 4 + qb
                    nc.vector.tensor_copy(
                        denw[:, p * QB : (p + 1) * QB], py[DH : DH + 1, :]
                    )
                    nc.vector.tensor_copy(yT[rb : rb + DH, qs], py[:DH, :])
                    nc.sync.dma_start(
                        den[p : p + 1, :], denw[:, p * QB : (p + 1) * QB]
                    )

                def den_prep(b):
                    """Reciprocal of the scattered denominators (fast approx)."""
                    _denw, den = den_all[b]
                    recf = dpool.tile([8, QB], F32, tag="recf", name=f"recf{b}")
                    nc.vector.reciprocal_approx_fast(recf[:], den[:])
                    rec = dpool.tile([8, QB], BF16, tag="rec", name=f"rec{b}")
                    nc.scalar.copy(rec[:], recf[:])
                    return rec

                def norm(b, rec):
                    cb = b * T
                    for qb in range(T // QB):
                        qs = slice(cb + qb * QB, cb + (qb + 1) * QB)
                        pb = p_psum.tile([128, 512], F32, tag="p", name=f"pb{b}_{qb}")
                        nc.tensor.matmul(
                            pb[:, :QB], emat[:, qb, :], rec[:],
                            start=True, stop=True,
                        )
                        nc.vector.tensor_mul(yT[:, qs], yT[:, qs], pb[:, :QB])

                def proj_quarter(b, i, deep=False):
                    """Output projection for 4 of the batch's 16 token tiles.

                    deep=True borrows the idle y_psum bank for a 3-deep psum
                    rotation (only safe when no sp unit is accumulating py).
                    """
                    cb = b * T
                    n = 0
                    for tt in range(i * 4, i * 4 + 4):
                        trow = cb + tt * 128
                        for half in range(2):
                            pool = y_psum if (deep and n % 3 == 2) else p_psum
                            tag = "py" if (deep and n % 3 == 2) else "p"
                            pp = pool.tile([128, 512], F32, tag=tag, name=f"pp{b}_{tt}_{half}")
                            n += 1
                            nc.tensor.matmul(
                                pp[:],
                                yT[:, trow : trow + 128],
                                wp_r[:, half * 512 : (half + 1) * 512],
                                start=True,
                                stop=True,
                            )
                            os_ = opool.tile([128, 512], BF16, tag="os", name=f"os{b}_{tt}_{half}")
                            # alternate copy engine: ACT carries exp, DVE the rest
                            if (tt + half) % 2 == 0:
                                nc.vector.tensor_copy(os_[:], pp[:])
                            else:
                                nc.scalar.copy(os_[:], pp[:])
                            nc.sync.dma_start(
                                out[trow : trow + 128, half * 512 : (half + 1) * 512],
                                os_[:],
                            )

                # ================= emission schedule =================
                # Per batch b the stream is: 8 sp units with fillers woven in
                # (next batch's qkv tiles + the previous batch's last two proj
                # quarters), then den reciprocal, next batch's V transposes
                # (fill the reciprocal's cross-engine latency), normalize, and
                # the first two proj quarters. The last two proj quarters ride
                # into the next batch's attention stream.
                dma_x(0)
                for tt in range(TPB):          # qkv for batch 0
                    qkv_tile(tt)
                for b in range(B):
                    denw = dpool.tile([1, 8 * QB], F32, tag="denw", name=f"denw{b}")
                    den = dpool.tile([8, QB], F32, tag="den", name=f"den{b}")
                    den_all[b] = (denw, den)
                    if b == 0:
                        vts_half(0, 0)
                        vts_half(0, 1)
                    sps = [(hl, qb) for qb in range(T // QB) for hl in range(HPC)]
                    fillers = []
                    if b > 0:
                        fillers.append(lambda bb=b - 1: proj_quarter(bb, 2))
                        fillers.append(lambda bb=b - 1: proj_quarter(bb, 3))
                    if b + 1 < B:
                        for j in range((b + 1) * TPB, (b + 2) * TPB):
                            fillers.append(lambda tt=j: qkv_tile(tt))
                    fpos = (
                        {0: 0, 1: 1, 2: 2, 3: 3, 4: 5, 5: 6}
                        if len(fillers) == 6
                        else {0: 1, 1: 3, 2: 5, 3: 6}
                    )
                    for i, (hl, qb) in enumerate(sps):
                        sp_unit(b, hl, qb)
                        for j, pos in fpos.items():
                            if pos == i and j < len(fillers):
                                fillers[j]()
                    rec = den_prep(b)
                    # next batch's V transposes run during the reciprocal chain
                    if b + 1 < B:
                        vts_half(b + 1, 0)
                    norm(b, rec)
                    proj_quarter(b, 0, deep=True)
                    if b + 1 < B:
                        vts_half(b + 1, 1)
                        vts_all.pop(b, None)
                    proj_quarter(b, 1, deep=True)
                    if b == B - 1:
                        proj_quarter(b, 2, deep=True)
                        proj_quarter(b, 3, deep=True)

    nc.compile()
    return nc


def _get_nc():
    global _CACHED_NC
    if _CACHED_NC is None:
        _CACHED_NC = _build()
    return _CACHED_NC


def kernel(x, W_qkv, b_qkv, W_proj, b_proj, _trace=False, _core_ids=None):
    global LAST_RESULT
    x = np.asarray(x, dtype=np.float32)
    W_qkv = np.asarray(W_qkv, dtype=np.float32)
    b_qkv = np.asarray(b_qkv, dtype=np.float32)
    W_proj = np.asarray(W_proj, dtype=np.float32)
    b_proj = np.asarray(b_proj, dtype=np.float32)

    xT = np.ascontiguousarray(x.reshape(NT, C).T).astype(BF16_NP)
    W_qkv_b = W_qkv.astype(BF16_NP)
    W_proj_b = W_proj.astype(BF16_NP)
    emat_np = np.zeros((8, 4, 128), dtype=BF16_NP)
    for qb in range(4):
        emat_np[qb, qb, :DH] = 1.0
        emat_np[4 + qb, qb, DH:] = 1.0
    core_ids = list(range(N_CORES)) if _core_ids is None else _core_ids

    def w_pcn(col0, col1):
        # [C, DPC] -> SBUF layout [p=128, c=CH, n=DPC], contiguous
        w = W_qkv_b[:, col0:col1].reshape(CH, 128, DPC)
        return np.ascontiguousarray(w.transpose(1, 0, 2))

    in_maps = []
    for core in range(len(core_ids)):
        s = slice(core * DPC, (core + 1) * DPC)
        in_maps.append(
            {
                "xT": xT,
                "wq": w_pcn(0 * C + core * DPC, 0 * C + (core + 1) * DPC),
                "wk": w_pcn(1 * C + core * DPC, 1 * C + (core + 1) * DPC),
                "wv": w_pcn(2 * C + core * DPC, 2 * C + (core + 1) * DPC),
                # device computes qT = psq*SCALE + bias, so pre-scale the q bias
                "bq": np.ascontiguousarray(b_qkv[0 * C + core * DPC : 0 * C + (core + 1) * DPC, None]) * np.float32(SCALE),
                "bk": np.ascontiguousarray(b_qkv[1 * C + core * DPC : 1 * C + (core + 1) * DPC, None]),
                "bv": np.ascontiguousarray(b_qkv[2 * C + core * DPC : 2 * C + (core + 1) * DPC, None]),
                "wp": np.ascontiguousarray(W_proj_b[s, :]),
                "emat": emat_np,
            }
        )

    nc = _get_nc()
    res = run_bass_kernel_spmd(nc, in_maps, core_ids, trace=_trace)
    LAST_RESULT = res

    acc = np.zeros((NT, C), dtype=np.float64)
    for r in res.results:
        acc += r["out"].astype(np.float64)
    acc += b_proj.astype(np.float64)
    return acc.reshape(B, T, C).astype(np.float32)


# revision 22
# speedup vs baseline: 1.1115x; 1.0379x over previous
"""Causal self-attention layer (B=4, T=2048, C=1024, H=16) on 8 TRN2 NeuronCores.

Sharding: Megatron-style tensor parallel over heads — 2 heads per core.
Each core computes q/k/v projections for its 2 heads, causal flash-style
attention with ones-columns on V to accumulate softmax denominators, and a
partial output projection against its 128-row slice of W_proj. The host sums
the 8 partial projections and adds b_proj.

All matmul operands are bfloat16 (pre-cast on host for x/weights; on-chip
activations write bf16 directly out of PSUM). fp32r matmuls run in
fp32_mode=HIGH which power-throttles the PE to 50% for most of the kernel;
bf16 keeps the PE mostly at full rate, halves the x/out DMA bytes, and
doubles DVE throughput on the mask multiplies.

Schedule: the q/k/v projection tiles for batch b+1 are interleaved into the
attention stream of batch b, so the PE never runs a long unbroken stream of
dense projection matmuls (which exhausts the power budget and triggers the
50%-duty throttle), and no phase boundary stalls the PE. V tiles are
transposed once per batch ([128,128] both-heads transpose) with two ones
columns (at free offsets 64 and 129) so each head's 65-wide PV lhsT slice
puts y in psum rows 0-63 and the softmax denominator in row 64. The V
transposes for batch b+1 are emitted in the denominator-reciprocal latency
gap of batch b. PSUM: 4 pools x 2 bufs x 2KB = exactly 8 banks.
"""
import sys

sys.path.insert(0, "/opt/trn_rl_repo")

import numpy as np
import ml_dtypes

import concourse.bass as bass  # noqa: F401
from concourse import bacc
import concourse.mybir as mybir
import concourse.tile as tile
from concourse.bass_utils import run_bass_kernel_spmd
from concourse.masks import make_identity

B, T, C = 4, 2048, 1024
H, DH = 16, 64
N_CORES = 8
HPC = H // N_CORES          # heads per core = 2
DPC = HPC * DH              # head-dims per core = 128
NT = B * T                  # 8192 tokens
CH = C // 128               # 8 contraction chunks
QB = 512                    # q-block width (moving dim)
KT = 128                    # k-tile width (PE partition dim)
CHUNK = 8                   # k-tiles per S/PV emission chunk
SCALE = 1.0 / 8.0           # 1/sqrt(DH)
TPB = T // QB               # qkv token tiles per batch = 4

F32 = mybir.dt.float32
BF16 = mybir.dt.bfloat16
AF = mybir.ActivationFunctionType
BF16_NP = ml_dtypes.bfloat16

_CACHED_NC = None
LAST_RESULT = None


def _build():
    nc = bacc.Bacc(None)

    xT = nc.dram_tensor("xT", [C, NT], BF16, kind="ExternalInput")
    # qkv weights pre-arranged on host to the SBUF layout [p, c, n]
    wq = nc.dram_tensor("wq", [128, CH, DPC], BF16, kind="ExternalInput")
    wk = nc.dram_tensor("wk", [128, CH, DPC], BF16, kind="ExternalInput")
    wv = nc.dram_tensor("wv", [128, CH, DPC], BF16, kind="ExternalInput")
    bqkv = nc.dram_tensor("bqkv", [DPC, 3], F32, kind="ExternalInput")
    wp = nc.dram_tensor("wp", [DPC, C], BF16, kind="ExternalInput")
    emat_in = nc.dram_tensor("emat", [8, 4, 128], BF16, kind="ExternalInput")
    out = nc.dram_tensor("out", [NT, C], BF16, kind="ExternalOutput")

    with tile.TileContext(nc) as tc:
        with (
            tc.tile_pool(name="const", bufs=1) as const,
            tc.tile_pool(name="res", bufs=1) as res,
        ):
            # --- constants (built in f32, cast to bf16 once) ---
            ident = const.tile([128, 128], BF16, tag="ident")
            # sliding causal mask: wmask[k, u] = 1 iff k <= u - 512; a crossing
            # tile r multiplies by wmask[:, 512-128r : 1024-128r]
            wmask = const.tile([128, 1024], BF16, tag="wmask")
            ones_col = const.tile([128, 1], BF16, tag="ones_col")
            # indicator lhsT per q-block: Emat[:, qb, j] selects den row qb
            # (head 0) for j<64 and row 4+qb (head 1) for j>=64, so one matmul
            # broadcasts both heads' reciprocals into a [128, 512] tile
            emat = const.tile([8, 4, 128], BF16, tag="emat")
            with tc.tile_pool(name="cstage", bufs=1) as cstage:
                ident_s = cstage.tile([128, 128], F32, tag="ident_s")
                make_identity(nc, ident_s[:])
                nc.vector.tensor_copy(ident[:], ident_s[:])

                wmask_s = cstage.tile([128, 1024], F32, tag="wmask_s")
                nc.gpsimd.memset(wmask_s[:], 0.0)
                nc.gpsimd.affine_select(
                    out=wmask_s[:],
                    in_=wmask_s[:],
                    compare_op=mybir.AluOpType.is_gt,
                    fill=1.0,
                    base=512,
                    # keep 0 where (512 + k - u) > 0, fill 1 where k <= u - 512
                    pattern=[[-1, 1024]],
                    channel_multiplier=1,
                )
                nc.vector.tensor_copy(wmask[:], wmask_s[:])

                ones_s = cstage.tile([128, 1], F32, tag="ones_s")
                nc.gpsimd.memset(ones_s[:], 1.0)
                nc.vector.tensor_copy(ones_col[:], ones_s[:])

            bqkv_t = const.tile([DPC, 3], F32, tag="bqkv")
            bq_t, bk_t, bv_t = bqkv_t[:, 0:1], bqkv_t[:, 1:2], bqkv_t[:, 2:3]

            # weights -> SBUF directly in bf16 (cast on host)
            wq_r = const.tile([128, CH, DPC], BF16, tag="wq_r")
            wk_r = const.tile([128, CH, DPC], BF16, tag="wk_r")
            wv_r = const.tile([128, CH, DPC], BF16, tag="wv_r")
            wp_r = const.tile([DPC, C], BF16, tag="wp_r")

            # --- residents ---
            qT = res.tile([DPC, NT], BF16, tag="qT")
            kT = res.tile([DPC, NT], BF16, tag="kT")
            vT = res.tile([DPC, NT], BF16, tag="vT")
            yT = res.tile([DPC, NT], BF16, tag="yT")

            xT_re = xT.rearrange("(c p) t -> p c t", p=128)
            n_ktiles = T // KT  # 16

            with (
                tc.tile_pool(name="xpool", bufs=3) as xpool,
                tc.tile_pool(name="vpool", bufs=34) as vpool,
                tc.tile_pool(name="epool", bufs=CHUNK + 3) as epool,
                tc.tile_pool(name="dpool", bufs=2) as dpool,
                tc.tile_pool(name="opool", bufs=6) as opool,
                tc.tile_pool(name="q_psum", bufs=2, space="PSUM") as q_psum,
                tc.tile_pool(name="s_psum", bufs=3, space="PSUM") as s_psum,
                tc.tile_pool(name="y_psum", bufs=1, space="PSUM") as y_psum,
                tc.tile_pool(name="p_psum", bufs=2, space="PSUM") as p_psum,
            ):
                xs_tiles = {}

                def dma_x(tt):
                    if tt >= NT // QB or tt in xs_tiles:
                        return
                    xs = xpool.tile([128, CH, QB], BF16, tag="xs", name=f"xs{tt}")
                    nc.sync.dma_start(
                        xs[:], xT_re[:, :, tt * QB : (tt + 1) * QB]
                    )
                    xs_tiles[tt] = xs

                def qkv_tile(tt):
                    """Project one 512-token tile into qT/kT/vT; prefetch x."""
                    dma_x(tt + 1)
                    xs = xs_tiles.pop(tt)
                    ts_ = slice(tt * QB, (tt + 1) * QB)
                    psq = q_psum.tile([128, QB], F32, tag="qkv", name=f"psq{tt}")
                    psk = q_psum.tile([128, QB], F32, tag="qkv", name=f"psk{tt}")
                    psv = q_psum.tile([128, QB], F32, tag="qkv", name=f"psv{tt}")
                    for ps, w_r in ((psq, wq_r), (psk, wk_r), (psv, wv_r)):
                        for c in range(CH):
                            nc.tensor.matmul(
                                ps[:], w_r[:, c, :], xs[:, c, :],
                                start=(c == 0), stop=(c == CH - 1),
                            )
                    # copy out of PSUM (+bias; q also scaled by 1/sqrt(dh))
                    nc.scalar.activation(qT[:, ts_], psq[:], AF.Identity, bias=bq_t[:], scale=SCALE)
                    nc.vector.tensor_scalar_add(kT[:, ts_], psk[:], bk_t[:])
                    nc.vector.tensor_scalar_add(vT[:, ts_], psv[:], bv_t[:])

                # per-batch state
                vts_all = {}   # b -> list of 16 [128, 130] tiles
                den_all = {}   # b -> (denw, den)

                def vts_half(b, half):
                    """Transpose 8 V token-tiles (both heads at once).

                    v tile layout [128 tok, 130]: cols 0-63 head0 dims, col 64
                    ones, cols 65-128 head1 dims, col 129 ones. Head hl's PV
                    lhsT is v[:, 65*hl : 65*hl+65] -> psum rows 0-63 = y,
                    row 64 = denominator.
                    """
                    cb = b * T
                    vts = vts_all.setdefault(b, [None] * n_ktiles)
                    for kt in range(half * 8, half * 8 + 8):
                        pt = s_psum.tile([128, 512], BF16, tag="s", name=f"pt{b}_{kt}")
                        nc.tensor.transpose(
                            pt[:, :128],
                            vT[:, cb + kt * KT : cb + (kt + 1) * KT],
                            ident[:],
                        )
                        v = vpool.tile([128, 130], BF16, tag="v", name=f"v{b}_{kt}")
                        nc.vector.tensor_copy(v[:, 0:64], pt[:, 0:64])
                        nc.scalar.copy(v[:, 65:129], pt[:, 64:128])
                        nc.vector.tensor_copy(v[:, 64:65], ones_col[:])
                        nc.vector.tensor_copy(v[:, 129:130], ones_col[:])
                        vts[kt] = v

                def sp_unit(b, hl, qb):
                    """Scores + exp + P@V for one (head, q-block)."""
                    cb = b * T
                    rb = hl * DH
                    vts = vts_all[b]
                    qs = slice(cb + qb * QB, cb + (qb + 1) * QB)
                    py = y_psum.tile([128, QB], F32, tag="py", name=f"py{b}_{hl}_{qb}")
                    nkt = (qb + 1) * (QB // KT)
                    for k0 in range(0, nkt, CHUNK):
                        kts = range(k0, min(k0 + CHUNK, nkt))
                        exs = {}
                        # scores + exp for this chunk
                        for kt in kts:
                            ps = s_psum.tile([128, QB], F32, tag="s", name=f"ps{kt}")
                            nc.tensor.matmul(
                                ps[:],
                                kT[rb : rb + DH, cb + kt * KT : cb + (kt + 1) * KT],
                                qT[rb : rb + DH, qs],
                                start=True,
                                stop=True,
                            )
                            ex = epool.tile([128, QB], BF16, tag="ex", name=f"ex{kt}")
                            nc.scalar.activation(ex[:], ps[:], AF.Exp)
                            r = kt - qb * (QB // KT)
                            if r >= 0:
                                # diagonal-crossing tile: zero out k > q
                                nc.vector.tensor_mul(
                                    ex[:], ex[:],
                                    wmask[:, 512 - r * KT : 1024 - r * KT],
                                )
                            exs[kt] = ex
                        # grouped P@V accumulation for this chunk
                        for kt in kts:
                            nc.tensor.matmul(
                                py[: DH + 1],
                                vts[kt][:, 65 * hl : 65 * hl + 65],
                                exs[kt][:],
                                start=(kt == 0),
                                stop=(kt == nkt - 1),
                            )
                    # stash unnormalized y; scatter the denominator row to its
                    # final partition right away (per-unit, off critical path)
                    denw, den = den_all[b]
                    p = hl * 4 + qb
                    nc.vector.tensor_copy(
                        denw[:, p * QB : (p + 1) * QB], py[DH : DH + 1, :]
                    )
                    nc.vector.tensor_copy(yT[rb : rb + DH, qs], py[:DH, :])
                    nc.sync.dma_start(
                        den[p : p + 1, :], denw[:, p * QB : (p + 1) * QB]
                    )

                def den_prep(b):
                    """Reciprocal of the scattered denominators (fast approx)."""
                    _denw, den = den_all[b]
                    recf = dpool.tile([8, QB], F32, tag="recf", name=f"recf{b}")
                    nc.vector.reciprocal_approx_fast(recf[:], den[:])
                    rec = dpool.tile([8, QB], BF16, tag="rec", name=f"rec{b}")
                    nc.scalar.copy(rec[:], recf[:])
                    return rec

                def norm(b, rec):
                    cb = b * T
                    for qb in range(T // QB):
                        qs = slice(cb + qb * QB, cb + (qb + 1) * QB)
                        pb = p_psum.tile([128, 512], F32, tag="p", name=f"pb{b}_{qb}")
                        nc.tensor.matmul(
                            pb[:, :QB], emat[:, qb, :], rec[:],
                            start=True, stop=True,
                        )
                        nc.vector.tensor_mul(yT[:, qs], yT[:, qs], pb[:, :QB])

                def proj_quarter(b, i, deep=False):
                    """Output projection for 4 of the batch's 16 token tiles.

                    deep=True borrows the idle y_psum bank for a 3-deep psum
                    rotation (only safe when no sp unit is accumulating py).
                    """
                    cb = b * T
                    n = 0
                    for tt in range(i * 4, i * 4 + 4):
                        trow = cb + tt * 128
                        for half in range(2):
                            pool = y_psum if (deep and n % 3 == 2) else p_psum
                            tag = "py" if (deep and n % 3 == 2) else "p"
                            pp = pool.tile([128, 512], F32, tag=tag, name=f"pp{b}_{tt}_{half}")
                            n += 1
                            nc.tensor.matmul(
                                pp[:],
                                yT[:, trow : trow + 128],
                                wp_r[:, half * 512 : (half + 1) * 512],
                                start=True,
                                stop=True,
                            )
                            os_ = opool.tile([128, 512], BF16, tag="os", name=f"os{b}_{tt}_{half}")
                            # alternate copy engine: ACT carries exp, DVE the rest
                            if (tt + half) % 2 == 0:
                                nc.vector.tensor_copy(os_[:], pp[:])
                            else:
                                nc.scalar.copy(os_[:], pp[:])
                            nc.sync.dma_start(
                                out[trow : trow + 128, half * 512 : (half + 1) * 512],
                                os_[:],
                            )

                # ================= emission schedule =================
                # Per batch b the stream is: 8 sp units with fillers woven in
                # (next batch's qkv tiles + the previous batch's last two proj
                # quarters), then den reciprocal, next batch's V transposes
                # (fill the reciprocal's cross-engine latency), normalize, and
                # the first two proj quarters. The last two proj quarters ride
                # into the next batch's attention stream.
                # DMA issue order: the first x tile and wq gate the first
                # matmul, so they go first (issues serialize at ~650ns each).
                dma_x(0)
                nc.sync.dma_start(wq_r[:], wq[:])
                nc.sync.dma_start(wk_r[:], wk[:])
                nc.sync.dma_start(wv_r[:], wv[:])
                nc.sync.dma_start(bqkv_t[:], bqkv[:])
                nc.sync.dma_start(wp_r[:], wp[:])
                nc.sync.dma_start(emat[:], emat_in[:])
                for tt in range(TPB):          # qkv for batch 0
                    qkv_tile(tt)
                for b in range(B):
                    denw = dpool.tile([1, 8 * QB], F32, tag="denw", name=f"denw{b}")
                    den = dpool.tile([8, QB], F32, tag="den", name=f"den{b}")
                    den_all[b] = (denw, den)
                    if b == 0:
                        vts_half(0, 0)
                        vts_half(0, 1)
                    sps = [(hl, qb) for qb in range(T // QB) for hl in range(HPC)]
                    fillers = []
                    if b > 0:
                        fillers.append(lambda bb=b - 1: proj_quarter(bb, 2))
                        fillers.append(lambda bb=b - 1: proj_quarter(bb, 3))
                    if b + 1 < B:
                        for j in range((b + 1) * TPB, (b + 2) * TPB):
                            fillers.append(lambda tt=j: qkv_tile(tt))
                    # pair bigger sp units with fillers so full-power units
                    # don't cluster at the batch boundary
                    fpos = (
                        {0: 1, 1: 2, 2: 3, 3: 4, 4: 5, 5: 6}
                        if len(fillers) != 4
                        else {0: 2, 1: 3, 2: 5, 3: 6}
                    )
                    for i, (hl, qb) in enumerate(sps):
                        sp_unit(b, hl, qb)
                        for j, pos in fpos.items():
                            if pos == i and j < len(fillers):
                                fillers[j]()
                    rec = den_prep(b)
                    # next batch's V transposes run during the reciprocal chain
                    if b + 1 < B:
                        vts_half(b + 1, 0)
                    norm(b, rec)
                    proj_quarter(b, 0, deep=True)
                    if b + 1 < B:
                        vts_half(b + 1, 1)
                        vts_all.pop(b, None)
                    proj_quarter(b, 1, deep=True)
                    if b == B - 1:
                        proj_quarter(b, 2, deep=True)
                        proj_quarter(b, 3, deep=True)

    nc.compile()
    return nc


def _get_nc():
    global _CACHED_NC
    if _CACHED_NC is None:
        _CACHED_NC = _build()
    return _CACHED_NC


def kernel(x, W_qkv, b_qkv, W_proj, b_proj, _trace=False, _core_ids=None):
    global LAST_RESULT
    x = np.asarray(x, dtype=np.float32)
    W_qkv = np.asarray(W_qkv, dtype=np.float32)
    b_qkv = np.asarray(b_qkv, dtype=np.float32)
    W_proj = np.asarray(W_proj, dtype=np.float32)
    b_proj = np.asarray(b_proj, dtype=np.float32)

    xT = np.ascontiguousarray(x.reshape(NT, C).T).astype(BF16_NP)
    W_qkv_b = W_qkv.astype(BF16_NP)
    W_proj_b = W_proj.astype(BF16_NP)
    emat_np = np.zeros((8, 4, 128), dtype=BF16_NP)
    for qb in range(4):
        emat_np[qb, qb, :DH] = 1.0
        emat_np[4 + qb, qb, DH:] = 1.0
    core_ids = list(range(N_CORES)) if _core_ids is None else _core_ids

    def w_pcn(col0, col1):
        # [C, DPC] -> SBUF layout [p=128, c=CH, n=DPC], contiguous
        w = W_qkv_b[:, col0:col1].reshape(CH, 128, DPC)
        return np.ascontiguousarray(w.transpose(1, 0, 2))

    in_maps = []
    for core in range(len(core_ids)):
        s = slice(core * DPC, (core + 1) * DPC)
        in_maps.append(
            {
                "xT": xT,
                "wq": w_pcn(0 * C + core * DPC, 0 * C + (core + 1) * DPC),
                "wk": w_pcn(1 * C + core * DPC, 1 * C + (core + 1) * DPC),
                "wv": w_pcn(2 * C + core * DPC, 2 * C + (core + 1) * DPC),
                # device computes qT = psq*SCALE + bias, so pre-scale the q bias
                "bqkv": np.ascontiguousarray(
                    np.stack(
                        [
                            b_qkv[0 * C + core * DPC : 0 * C + (core + 1) * DPC] * np.float32(SCALE),
                            b_qkv[1 * C + core * DPC : 1 * C + (core + 1) * DPC],
                            b_qkv[2 * C + core * DPC : 2 * C + (core + 1) * DPC],
                        ],
                        axis=1,
                    )
                ),
                "wp": np.ascontiguousarray(W_proj_b[s, :]),
                "emat": emat_np,
            }
        )

    nc = _get_nc()
    res = run_bass_kernel_spmd(nc, in_maps, core_ids, trace=_trace)
    LAST_RESULT = res

    acc = np.zeros((NT, C), dtype=np.float64)
    for r in res.results:
        acc += r["out"].astype(np.float64)
    acc += b_proj.astype(np.float64)
    return acc.reshape(B, T, C).astype(np.float32)


# revision 31
# speedup vs baseline: 1.1308x; 1.0174x over previous
"""Causal self-attention layer (B=4, T=2048, C=1024, H=16) on 8 TRN2 NeuronCores.

Sharding: Megatron-style tensor parallel over heads — 2 heads per core.
Each core computes q/k/v projections for its 2 heads, causal flash-style
attention with ones-columns on V to accumulate softmax denominators, and a
partial output projection against its 128-row slice of W_proj. The host sums
the 8 partial projections and adds b_proj.

All matmul operands are bfloat16 (pre-cast on host for x/weights; on-chip
activations write bf16 directly out of PSUM). fp32r matmuls run in
fp32_mode=HIGH which power-throttles the PE to 50% for most of the kernel;
bf16 keeps the PE mostly at full rate, halves the x/out DMA bytes, and
doubles DVE throughput on the mask multiplies.

Schedule: the q/k/v projection tiles for batch b+1 are interleaved into the
attention stream of batch b, so the PE never runs a long unbroken stream of
dense projection matmuls (which exhausts the power budget and triggers the
50%-duty throttle), and no phase boundary stalls the PE. V tiles are
transposed once per batch ([128,128] both-heads transpose) with two ones
columns (at free offsets 64 and 129) so each head's 65-wide PV lhsT slice
puts y in psum rows 0-63 and the softmax denominator in row 64. The V
transposes for batch b+1 are emitted in the denominator-reciprocal latency
gap of batch b. PSUM: 4 pools x 2 bufs x 2KB = exactly 8 banks.
"""
import sys

sys.path.insert(0, "/opt/trn_rl_repo")

import numpy as np
import ml_dtypes

import concourse.bass as bass  # noqa: F401
from concourse import bacc
import concourse.mybir as mybir
import concourse.tile as tile
from concourse.bass_utils import run_bass_kernel_spmd
from concourse.masks import make_identity

B, T, C = 4, 2048, 1024
H, DH = 16, 64
N_CORES = 8
HPC = H // N_CORES          # heads per core = 2
DPC = HPC * DH              # head-dims per core = 128
NT = B * T                  # 8192 tokens
CH = C // 128               # 8 contraction chunks
QB = 512                    # q-block width (moving dim)
KT = 128                    # k-tile width (PE partition dim)
CHUNK = 8                   # k-tiles per S/PV emission chunk
SCALE = 1.0 / 8.0           # 1/sqrt(DH)
TPB = T // QB               # qkv token tiles per batch = 4

F32 = mybir.dt.float32
BF16 = mybir.dt.bfloat16
AF = mybir.ActivationFunctionType
BF16_NP = ml_dtypes.bfloat16

_CACHED_NC = None
LAST_RESULT = None


def _build():
    nc = bacc.Bacc(None)

    xT = nc.dram_tensor("xT", [C, NT], BF16, kind="ExternalInput")
    # qkv weights pre-arranged on host to the SBUF layout [p, c, n]
    wq = nc.dram_tensor("wq", [128, CH, DPC], BF16, kind="ExternalInput")
    wk = nc.dram_tensor("wk", [128, CH, DPC], BF16, kind="ExternalInput")
    wv = nc.dram_tensor("wv", [128, CH, DPC], BF16, kind="ExternalInput")
    bqkv = nc.dram_tensor("bqkv", [DPC, 3], F32, kind="ExternalInput")
    wp = nc.dram_tensor("wp", [DPC, C], BF16, kind="ExternalInput")
    # head selector: esel[0,:64]=1, esel[1,64:]=1 — broadcasts a [2,512]
    # reciprocal pair into a [128,512] per-dim normalizer via a K=2 matmul
    esel_in = nc.dram_tensor("esel", [2, 128], BF16, kind="ExternalInput")
    out = nc.dram_tensor("out", [NT, C], BF16, kind="ExternalOutput")

    with tile.TileContext(nc) as tc:
        with (
            tc.tile_pool(name="const", bufs=1) as const,
            tc.tile_pool(name="res", bufs=1) as res,
        ):
            # --- constants (built in f32, cast to bf16 once) ---
            ident = const.tile([128, 128], BF16, tag="ident")
            # sliding causal mask: wmask[k, u] = 1 iff k <= u - 512; a crossing
            # tile r multiplies by wmask[:, 512-128r : 1024-128r]
            wmask = const.tile([128, 1024], BF16, tag="wmask")
            ones_col = const.tile([128, 1], BF16, tag="ones_col")
            esel = const.tile([2, 128], BF16, tag="esel")
            with tc.tile_pool(name="cstage", bufs=1) as cstage:
                ident_s = cstage.tile([128, 128], F32, tag="ident_s")
                make_identity(nc, ident_s[:])
                nc.vector.tensor_copy(ident[:], ident_s[:])

                wmask_s = cstage.tile([128, 1024], F32, tag="wmask_s")
                nc.gpsimd.memset(wmask_s[:], 0.0)
                nc.gpsimd.affine_select(
                    out=wmask_s[:],
                    in_=wmask_s[:],
                    compare_op=mybir.AluOpType.is_gt,
                    fill=1.0,
                    base=512,
                    # keep 0 where (512 + k - u) > 0, fill 1 where k <= u - 512
                    pattern=[[-1, 1024]],
                    channel_multiplier=1,
                )
                nc.vector.tensor_copy(wmask[:], wmask_s[:])

                ones_s = cstage.tile([128, 1], F32, tag="ones_s")
                nc.gpsimd.memset(ones_s[:], 1.0)
                nc.vector.tensor_copy(ones_col[:], ones_s[:])

            bqkv_t = const.tile([DPC, 3], F32, tag="bqkv")
            bq_t, bk_t, bv_t = bqkv_t[:, 0:1], bqkv_t[:, 1:2], bqkv_t[:, 2:3]

            # weights -> SBUF directly in bf16 (cast on host)
            wq_r = const.tile([128, CH, DPC], BF16, tag="wq_r")
            wk_r = const.tile([128, CH, DPC], BF16, tag="wk_r")
            wv_r = const.tile([128, CH, DPC], BF16, tag="wv_r")
            wp_r = const.tile([DPC, C], BF16, tag="wp_r")

            # --- residents ---
            qT = res.tile([DPC, NT], BF16, tag="qT")
            kT = res.tile([DPC, NT], BF16, tag="kT")
            vT = res.tile([DPC, NT], BF16, tag="vT")
            yT = res.tile([DPC, NT], BF16, tag="yT")

            xT_re = xT.rearrange("(c p) t -> p c t", p=128)
            n_ktiles = T // KT  # 16

            with (
                tc.tile_pool(name="xpool", bufs=3) as xpool,
                tc.tile_pool(name="vpool", bufs=34) as vpool,
                tc.tile_pool(name="epool", bufs=CHUNK + 3) as epool,
                tc.tile_pool(name="dpool", bufs=2) as dpool,
                tc.tile_pool(name="opool", bufs=6) as opool,
                tc.tile_pool(name="q_psum", bufs=2, space="PSUM") as q_psum,
                tc.tile_pool(name="s_psum", bufs=3, space="PSUM") as s_psum,
                tc.tile_pool(name="y_psum", bufs=1, space="PSUM") as y_psum,
                tc.tile_pool(name="p_psum", bufs=2, space="PSUM") as p_psum,
            ):
                xs_tiles = {}

                def dma_x(tt):
                    if tt >= NT // QB or tt in xs_tiles:
                        return
                    xs = xpool.tile([128, CH, QB], BF16, tag="xs", name=f"xs{tt}")
                    nc.sync.dma_start(
                        xs[:], xT_re[:, :, tt * QB : (tt + 1) * QB]
                    )
                    xs_tiles[tt] = xs

                def qkv_tile(tt):
                    """Project one 512-token tile into qT/kT/vT; prefetch x."""
                    dma_x(tt + 1)
                    xs = xs_tiles.pop(tt)
                    ts_ = slice(tt * QB, (tt + 1) * QB)
                    psq = q_psum.tile([128, QB], F32, tag="qkv", name=f"psq{tt}")
                    psk = q_psum.tile([128, QB], F32, tag="qkv", name=f"psk{tt}")
                    psv = q_psum.tile([128, QB], F32, tag="qkv", name=f"psv{tt}")
                    for ps, w_r in ((psq, wq_r), (psk, wk_r), (psv, wv_r)):
                        for c in range(CH):
                            nc.tensor.matmul(
                                ps[:], w_r[:, c, :], xs[:, c, :],
                                start=(c == 0), stop=(c == CH - 1),
                            )
                    # copy out of PSUM (+bias; q also scaled by 1/sqrt(dh))
                    nc.scalar.activation(qT[:, ts_], psq[:], AF.Identity, bias=bq_t[:], scale=SCALE)
                    nc.vector.tensor_scalar_add(kT[:, ts_], psk[:], bk_t[:])
                    nc.vector.tensor_scalar_add(vT[:, ts_], psv[:], bv_t[:])

                # per-batch state
                vts_all = {}   # b -> list of 16 [128, 130] tiles
                den_all = {}   # b -> (denw, den)

                def vts_half(b, half):
                    """Transpose 8 V token-tiles (both heads at once).

                    v tile layout [128 tok, 130]: cols 0-63 head0 dims, col 64
                    ones, cols 65-128 head1 dims, col 129 ones. Head hl's PV
                    lhsT is v[:, 65*hl : 65*hl+65] -> psum rows 0-63 = y,
                    row 64 = denominator.
                    """
                    cb = b * T
                    vts = vts_all.setdefault(b, [None] * n_ktiles)
                    for kt in range(half * 8, half * 8 + 8):
                        pt = s_psum.tile([128, 512], BF16, tag="s", name=f"pt{b}_{kt}")
                        nc.tensor.transpose(
                            pt[:, :128],
                            vT[:, cb + kt * KT : cb + (kt + 1) * KT],
                            ident[:],
                        )
                        v = vpool.tile([128, 130], BF16, tag="v", name=f"v{b}_{kt}")
                        nc.vector.tensor_copy(v[:, 0:64], pt[:, 0:64])
                        nc.scalar.copy(v[:, 65:129], pt[:, 64:128])
                        nc.vector.tensor_copy(v[:, 64:65], ones_col[:])
                        nc.vector.tensor_copy(v[:, 129:130], ones_col[:])
                        vts[kt] = v

                def sp_unit(b, hl, qb):
                    """Scores + exp + P@V for one (head, q-block)."""
                    cb = b * T
                    rb = hl * DH
                    vts = vts_all[b]
                    qs = slice(cb + qb * QB, cb + (qb + 1) * QB)
                    py = y_psum.tile([128, QB], F32, tag="py", name=f"py{b}_{hl}_{qb}")
                    nkt = (qb + 1) * (QB // KT)
                    for k0 in range(0, nkt, CHUNK):
                        kts = range(k0, min(k0 + CHUNK, nkt))
                        exs = {}
                        # scores + exp for this chunk
                        for kt in kts:
                            ps = s_psum.tile([128, QB], F32, tag="s", name=f"ps{kt}")
                            nc.tensor.matmul(
                                ps[:],
                                kT[rb : rb + DH, cb + kt * KT : cb + (kt + 1) * KT],
                                qT[rb : rb + DH, qs],
                                start=True,
                                stop=True,
                            )
                            ex = epool.tile([128, QB], BF16, tag="ex", name=f"ex{kt}")
                            nc.scalar.activation(ex[:], ps[:], AF.Exp)
                            r = kt - qb * (QB // KT)
                            if r >= 0:
                                # diagonal-crossing tile: zero out k > q
                                nc.vector.tensor_mul(
                                    ex[:], ex[:],
                                    wmask[:, 512 - r * KT : 1024 - r * KT],
                                )
                            exs[kt] = ex
                        # grouped P@V accumulation for this chunk
                        for kt in kts:
                            nc.tensor.matmul(
                                py[: DH + 1],
                                vts[kt][:, 65 * hl : 65 * hl + 65],
                                exs[kt][:],
                                start=(kt == 0),
                                stop=(kt == nkt - 1),
                            )
                    # stash unnormalized y; scatter the denominator row into
                    # this q-block's [2, QB] pair tile (DMA may write any
                    # partition; compute engines may not)
                    denw, dens = den_all[b]
                    p = 2 * qb + hl
                    nc.vector.tensor_copy(
                        denw[:, p * QB : (p + 1) * QB], py[DH : DH + 1, :]
                    )
                    nc.vector.tensor_copy(yT[rb : rb + DH, qs], py[:DH, :])
                    nc.sync.dma_start(
                        dens[qb][hl : hl + 1, :], denw[:, p * QB : (p + 1) * QB]
                    )

                def norm_qb(b, qb):
                    """Reciprocal + normalize for one q-block (both heads)."""
                    _denw, dens = den_all[b]
                    cb = b * T
                    qs = slice(cb + qb * QB, cb + (qb + 1) * QB)
                    recf = dpool.tile([2, QB], F32, tag="recf", name=f"recf{b}_{qb}")
                    nc.vector.reciprocal_approx_fast(recf[:], dens[qb][:])
                    rec = dpool.tile([2, QB], BF16, tag="rec", name=f"rec{b}_{qb}")
                    nc.scalar.copy(rec[:], recf[:])
                    pb = p_psum.tile([128, 512], F32, tag="p", name=f"pb{b}_{qb}")
                    nc.tensor.matmul(
                        pb[:, :QB], esel[:], rec[:],
                        start=True, stop=True,
                    )
                    nc.vector.tensor_mul(yT[:, qs], yT[:, qs], pb[:, :QB])

                def proj_quarter(b, i, deep=False):
                    """Output projection for 4 of the batch's 16 token tiles.

                    deep=True borrows the idle y_psum bank for a 3-deep psum
                    rotation (only safe when no sp unit is accumulating py).
                    """
                    cb = b * T
                    n = 0
                    for tt in range(i * 4, i * 4 + 4):
                        trow = cb + tt * 128
                        for half in range(2):
                            pool = y_psum if (deep and n % 3 == 2) else p_psum
                            tag = "py" if (deep and n % 3 == 2) else "p"
                            pp = pool.tile([128, 512], F32, tag=tag, name=f"pp{b}_{tt}_{half}")
                            n += 1
                            nc.tensor.matmul(
                                pp[:],
                                yT[:, trow : trow + 128],
                                wp_r[:, half * 512 : (half + 1) * 512],
                                start=True,
                                stop=True,
                            )
                            os_ = opool.tile([128, 512], BF16, tag="os", name=f"os{b}_{tt}_{half}")
                            # alternate copy engine: ACT carries exp, DVE the rest
                            if (tt + half) % 2 == 0:
                                nc.vector.tensor_copy(os_[:], pp[:])
                            else:
                                nc.scalar.copy(os_[:], pp[:])
                            nc.sync.dma_start(
                                out[trow : trow + 128, half * 512 : (half + 1) * 512],
                                os_[:],
                            )

                # ================= emission schedule =================
                # Per batch b: 8 sp units (hl pairs per q-block). After each
                # pair, that q-block is normalized (per-qb reciprocal via a
                # K=2 selector matmul). ALL proj quarters are carried into the
                # next batch's attention stream as fillers, woven 1:1 with the
                # next batch's qkv tiles, so full-power matmul runs never
                # cluster and the serial tail is one quarter. The V transposes
                # for batch b+1 form the (low-power) end block.
                # DMA issue order: the first x tile and wq gate the first
                # matmul, so they go first (issues serialize at ~650ns each).
                dma_x(0)
                nc.sync.dma_start(wq_r[:], wq[:])
                nc.sync.dma_start(wk_r[:], wk[:])
                nc.sync.dma_start(wv_r[:], wv[:])
                nc.sync.dma_start(bqkv_t[:], bqkv[:])
                nc.sync.dma_start(wp_r[:], wp[:])
                nc.sync.dma_start(esel[:], esel_in[:])
                for tt in range(TPB):          # qkv for batch 0
                    qkv_tile(tt)
                for b in range(B):
                    denw = dpool.tile([1, 8 * QB], F32, tag="denw", name=f"denw{b}")
                    dens = [
                        dpool.tile([2, QB], F32, tag="den", name=f"den{b}_{qb}")
                        for qb in range(T // QB)
                    ]
                    den_all[b] = (denw, dens)
                    if b == 0:
                        vts_half(0, 0)
                        vts_half(0, 1)
                    # fillers[i] runs after sp unit i: previous batch's proj
                    # quarters and next batch's qkv tiles, alternating
                    fillers = {}
                    if b == 0:
                        for j, pos in enumerate((2, 3, 5, 6)):
                            fillers[pos] = lambda tt=TPB + j: qkv_tile(tt)
                    elif b < B - 1:
                        fillers = {
                            0: lambda: proj_quarter(b - 1, 0),
                            1: lambda: qkv_tile((b + 1) * TPB + 0),
                            2: lambda: proj_quarter(b - 1, 1),
                            3: lambda: qkv_tile((b + 1) * TPB + 1),
                            4: lambda: proj_quarter(b - 1, 2),
                            5: lambda: qkv_tile((b + 1) * TPB + 2),
                            6: lambda: proj_quarter(b - 1, 3),
                            7: lambda: qkv_tile((b + 1) * TPB + 3),
                        }
                    else:
                        # last batch: weave in b-1's quarters and our own
                        # earlier-normalized quarters
                        fillers = {
                            0: lambda: proj_quarter(b - 1, 0),
                            2: lambda: proj_quarter(b - 1, 1),
                            3: lambda: proj_quarter(b, 0),
                            4: lambda: proj_quarter(b - 1, 2),
                            5: lambda: proj_quarter(b, 1),
                            6: lambda: proj_quarter(b - 1, 3),
                            7: lambda: proj_quarter(b, 2),
                        }
                    sps = [(hl, qb) for qb in range(T // QB) for hl in range(HPC)]
                    for i, (hl, qb) in enumerate(sps):
                        sp_unit(b, hl, qb)
                        if hl == HPC - 1:
                            norm_qb(b, qb)
                        f = fillers.get(i)
                        if f is not None:
                            f()
                    if b + 1 < B:
                        vts_half(b + 1, 0)
                        vts_half(b + 1, 1)
                        vts_all.pop(b, None)
                    else:
                        proj_quarter(b, 3, deep=True)

    nc.compile()
    return nc


def _get_nc():
    global _CACHED_NC
    if _CACHED_NC is None:
        _CACHED_NC = _build()
    return _CACHED_NC


def kernel(x, W_qkv, b_qkv, W_proj, b_proj, _trace=False, _core_ids=None):
    global LAST_RESULT
    x = np.asarray(x, dtype=np.float32)
    W_qkv = np.asarray(W_qkv, dtype=np.float32)
    b_qkv = np.asarray(b_qkv, dtype=np.float32)
    W_proj = np.asarray(W_proj, dtype=np.float32)
    b_proj = np.asarray(b_proj, dtype=np.float32)

    xT = np.ascontiguousarray(x.reshape(NT, C).T).astype(BF16_NP)
    W_qkv_b = W_qkv.astype(BF16_NP)
    W_proj_b = W_proj.astype(BF16_NP)
    esel_np = np.zeros((2, 128), dtype=BF16_NP)
    esel_np[0, :DH] = 1.0
    esel_np[1, DH:] = 1.0
    core_ids = list(range(N_CORES)) if _core_ids is None else _core_ids

    def w_pcn(col0, col1):
        # [C, DPC] -> SBUF layout [p=128, c=CH, n=DPC], contiguous
        w = W_qkv_b[:, col0:col1].reshape(CH, 128, DPC)
        return np.ascontiguousarray(w.transpose(1, 0, 2))

    in_maps = []
    for core in range(len(core_ids)):
        s = slice(core * DPC, (core + 1) * DPC)
        in_maps.append(
            {
                "xT": xT,
                "wq": w_pcn(0 * C + core * DPC, 0 * C + (core + 1) * DPC),
                "wk": w_pcn(1 * C + core * DPC, 1 * C + (core + 1) * DPC),
                "wv": w_pcn(2 * C + core * DPC, 2 * C + (core + 1) * DPC),
                # device computes qT = psq*SCALE + bias, so pre-scale the q bias
                "bqkv": np.ascontiguousarray(
                    np.stack(
                        [
                            b_qkv[0 * C + core * DPC : 0 * C + (core + 1) * DPC] * np.float32(SCALE),
                            b_qkv[1 * C + core * DPC : 1 * C + (core + 1) * DPC],
                            b_qkv[2 * C + core * DPC : 2 * C + (core + 1) * DPC],
                        ],
                        axis=1,
                    )
                ),
                "wp": np.ascontiguousarray(W_proj_b[s, :]),
                "esel": esel_np,
            }
        )

    nc = _get_nc()
    res = run_bass_kernel_spmd(nc, in_maps, core_ids, trace=_trace)
    LAST_RESULT = res

    acc = np.zeros((NT, C), dtype=np.float64)
    for r in res.results:
        acc += r["out"].astype(np.float64)
    acc += b_proj.astype(np.float64)
    return acc.reshape(B, T, C).astype(np.float32)


# revision 34
# speedup vs baseline: 1.1332x; 1.0021x over previous
"""Causal self-attention layer (B=4, T=2048, C=1024, H=16) on 8 TRN2 NeuronCores.

Sharding: Megatron-style tensor parallel over heads — 2 heads per core.
Each core computes q/k/v projections for its 2 heads, causal flash-style
attention with ones-columns on V to accumulate softmax denominators, and a
partial output projection against its 128-row slice of W_proj. The host sums
the 8 partial projections and adds b_proj.

All matmul operands are bfloat16 (pre-cast on host for x/weights; on-chip
activations write bf16 directly out of PSUM). fp32r matmuls run in
fp32_mode=HIGH which power-throttles the PE to 50% for most of the kernel;
bf16 keeps the PE mostly at full rate, halves the x/out DMA bytes, and
doubles DVE throughput on the mask multiplies.

Schedule: the q/k/v projection tiles for batch b+1 are interleaved into the
attention stream of batch b, so the PE never runs a long unbroken stream of
dense projection matmuls (which exhausts the power budget and triggers the
50%-duty throttle), and no phase boundary stalls the PE. V tiles are
transposed once per batch ([128,128] both-heads transpose) with two ones
columns (at free offsets 64 and 129) so each head's 65-wide PV lhsT slice
puts y in psum rows 0-63 and the softmax denominator in row 64. The V
transposes for batch b+1 are emitted in the denominator-reciprocal latency
gap of batch b. PSUM: 4 pools x 2 bufs x 2KB = exactly 8 banks.
"""
import sys

sys.path.insert(0, "/opt/trn_rl_repo")

import numpy as np
import ml_dtypes

import concourse.bass as bass  # noqa: F401
from concourse import bacc
import concourse.mybir as mybir
import concourse.tile as tile
from concourse.bass_utils import run_bass_kernel_spmd
from concourse.masks import make_identity

B, T, C = 4, 2048, 1024
H, DH = 16, 64
N_CORES = 8
HPC = H // N_CORES          # heads per core = 2
DPC = HPC * DH              # head-dims per core = 128
NT = B * T                  # 8192 tokens
CH = C // 128               # 8 contraction chunks
QB = 512                    # q-block width (moving dim)
KT = 128                    # k-tile width (PE partition dim)
CHUNK = 8                   # k-tiles per S/PV emission chunk
SCALE = 1.0 / 8.0           # 1/sqrt(DH)
TPB = T // QB               # qkv token tiles per batch = 4

F32 = mybir.dt.float32
BF16 = mybir.dt.bfloat16
AF = mybir.ActivationFunctionType
BF16_NP = ml_dtypes.bfloat16

_CACHED_NC = None
LAST_RESULT = None


def _build():
    nc = bacc.Bacc(None)

    xT = nc.dram_tensor("xT", [C, NT], BF16, kind="ExternalInput")
    # qkv weights pre-arranged on host to the SBUF layout [p, c, n]
    wq = nc.dram_tensor("wq", [128, CH, DPC], BF16, kind="ExternalInput")
    wk = nc.dram_tensor("wk", [128, CH, DPC], BF16, kind="ExternalInput")
    wv = nc.dram_tensor("wv", [128, CH, DPC], BF16, kind="ExternalInput")
    bqkv = nc.dram_tensor("bqkv", [DPC, 3], F32, kind="ExternalInput")
    wp = nc.dram_tensor("wp", [DPC, C], BF16, kind="ExternalInput")
    # head selector: esel[0,:64]=1, esel[1,64:]=1 — broadcasts a [2,512]
    # reciprocal pair into a [128,512] per-dim normalizer via a K=2 matmul
    esel_in = nc.dram_tensor("esel", [2, 128], BF16, kind="ExternalInput")
    out = nc.dram_tensor("out", [NT, C], BF16, kind="ExternalOutput")

    with tile.TileContext(nc) as tc:
        with (
            tc.tile_pool(name="const", bufs=1) as const,
            tc.tile_pool(name="res", bufs=1) as res,
        ):
            # --- constants (built in f32, cast to bf16 once) ---
            ident = const.tile([128, 128], BF16, tag="ident")
            # sliding causal mask: wmask[k, u] = 1 iff k <= u - 512; a crossing
            # tile r multiplies by wmask[:, 512-128r : 1024-128r]
            wmask = const.tile([128, 1024], BF16, tag="wmask")
            ones_col = const.tile([128, 1], BF16, tag="ones_col")
            esel = const.tile([2, 128], BF16, tag="esel")
            with tc.tile_pool(name="cstage", bufs=1) as cstage:
                ident_s = cstage.tile([128, 128], F32, tag="ident_s")
                make_identity(nc, ident_s[:])
                nc.vector.tensor_copy(ident[:], ident_s[:])

                wmask_s = cstage.tile([128, 1024], F32, tag="wmask_s")
                nc.gpsimd.memset(wmask_s[:], 0.0)
                nc.gpsimd.affine_select(
                    out=wmask_s[:],
                    in_=wmask_s[:],
                    compare_op=mybir.AluOpType.is_gt,
                    fill=1.0,
                    base=512,
                    # keep 0 where (512 + k - u) > 0, fill 1 where k <= u - 512
                    pattern=[[-1, 1024]],
                    channel_multiplier=1,
                )
                nc.vector.tensor_copy(wmask[:], wmask_s[:])

                ones_s = cstage.tile([128, 1], F32, tag="ones_s")
                nc.gpsimd.memset(ones_s[:], 1.0)
                nc.vector.tensor_copy(ones_col[:], ones_s[:])

            bqkv_t = const.tile([DPC, 3], F32, tag="bqkv")
            bq_t, bk_t, bv_t = bqkv_t[:, 0:1], bqkv_t[:, 1:2], bqkv_t[:, 2:3]

            # weights -> SBUF directly in bf16 (cast on host)
            wq_r = const.tile([128, CH, DPC], BF16, tag="wq_r")
            wk_r = const.tile([128, CH, DPC], BF16, tag="wk_r")
            wv_r = const.tile([128, CH, DPC], BF16, tag="wv_r")
            wp_r = const.tile([DPC, C], BF16, tag="wp_r")

            # --- residents ---
            qT = res.tile([DPC, NT], BF16, tag="qT")
            kT = res.tile([DPC, NT], BF16, tag="kT")
            vT = res.tile([DPC, NT], BF16, tag="vT")
            yT = res.tile([DPC, NT], BF16, tag="yT")

            xT_re = xT.rearrange("(c p) t -> p c t", p=128)
            n_ktiles = T // KT  # 16

            with (
                tc.tile_pool(name="xpool", bufs=3) as xpool,
                tc.tile_pool(name="vpool", bufs=34) as vpool,
                tc.tile_pool(name="epool", bufs=CHUNK + 3) as epool,
                tc.tile_pool(name="dpool", bufs=2) as dpool,
                tc.tile_pool(name="opool", bufs=6) as opool,
                tc.tile_pool(name="q_psum", bufs=2, space="PSUM") as q_psum,
                tc.tile_pool(name="s_psum", bufs=3, space="PSUM") as s_psum,
                tc.tile_pool(name="y_psum", bufs=1, space="PSUM") as y_psum,
                tc.tile_pool(name="p_psum", bufs=2, space="PSUM") as p_psum,
            ):
                xs_tiles = {}

                def dma_x(tt):
                    if tt >= NT // QB or tt in xs_tiles:
                        return
                    xs = xpool.tile([128, CH, QB], BF16, tag="xs", name=f"xs{tt}")
                    if tt == 0:
                        # split the first tile across 4 DMA queues — it gates
                        # the very first matmul (a single queue moves ~110GB/s)
                        for c0 in range(0, CH, 2):
                            nc.sync.dma_start(
                                xs[:, c0 : c0 + 2, :],
                                xT_re[:, c0 : c0 + 2, tt * QB : (tt + 1) * QB],
                            )
                    else:
                        nc.sync.dma_start(
                            xs[:], xT_re[:, :, tt * QB : (tt + 1) * QB]
                        )
                    xs_tiles[tt] = xs

                def qkv_tile(tt):
                    """Project one 512-token tile into qT/kT/vT; prefetch x."""
                    dma_x(tt + 1)
                    xs = xs_tiles.pop(tt)
                    ts_ = slice(tt * QB, (tt + 1) * QB)
                    psq = q_psum.tile([128, QB], F32, tag="qkv", name=f"psq{tt}")
                    psk = q_psum.tile([128, QB], F32, tag="qkv", name=f"psk{tt}")
                    psv = q_psum.tile([128, QB], F32, tag="qkv", name=f"psv{tt}")
                    for ps, w_r in ((psq, wq_r), (psk, wk_r), (psv, wv_r)):
                        for c in range(CH):
                            nc.tensor.matmul(
                                ps[:], w_r[:, c, :], xs[:, c, :],
                                start=(c == 0), stop=(c == CH - 1),
                            )
                    # copy out of PSUM (+bias; q also scaled by 1/sqrt(dh))
                    nc.scalar.activation(qT[:, ts_], psq[:], AF.Identity, bias=bq_t[:], scale=SCALE)
                    nc.vector.tensor_scalar_add(kT[:, ts_], psk[:], bk_t[:])
                    nc.vector.tensor_scalar_add(vT[:, ts_], psv[:], bv_t[:])

                # per-batch state
                vts_all = {}   # b -> list of 16 [128, 130] tiles
                den_all = {}   # b -> (denw, den)

                def vts_half(b, half):
                    """Transpose 8 V token-tiles (both heads at once).

                    v tile layout [128 tok, 130]: cols 0-63 head0 dims, col 64
                    ones, cols 65-128 head1 dims, col 129 ones. Head hl's PV
                    lhsT is v[:, 65*hl : 65*hl+65] -> psum rows 0-63 = y,
                    row 64 = denominator.
                    """
                    cb = b * T
                    vts = vts_all.setdefault(b, [None] * n_ktiles)
                    for kt in range(half * 8, half * 8 + 8):
                        pt = s_psum.tile([128, 512], BF16, tag="s", name=f"pt{b}_{kt}")
                        nc.tensor.transpose(
                            pt[:, :128],
                            vT[:, cb + kt * KT : cb + (kt + 1) * KT],
                            ident[:],
                        )
                        v = vpool.tile([128, 130], BF16, tag="v", name=f"v{b}_{kt}")
                        nc.vector.tensor_copy(v[:, 0:64], pt[:, 0:64])
                        nc.scalar.copy(v[:, 65:129], pt[:, 64:128])
                        nc.vector.tensor_copy(v[:, 64:65], ones_col[:])
                        nc.vector.tensor_copy(v[:, 129:130], ones_col[:])
                        vts[kt] = v

                def sp_unit(b, hl, qb):
                    """Scores + exp + P@V for one (head, q-block)."""
                    cb = b * T
                    rb = hl * DH
                    vts = vts_all[b]
                    qs = slice(cb + qb * QB, cb + (qb + 1) * QB)
                    py = y_psum.tile([128, QB], F32, tag="py", name=f"py{b}_{hl}_{qb}")
                    nkt = (qb + 1) * (QB // KT)
                    for k0 in range(0, nkt, CHUNK):
                        kts = range(k0, min(k0 + CHUNK, nkt))
                        exs = {}
                        # scores + exp for this chunk
                        for kt in kts:
                            ps = s_psum.tile([128, QB], F32, tag="s", name=f"ps{kt}")
                            nc.tensor.matmul(
                                ps[:],
                                kT[rb : rb + DH, cb + kt * KT : cb + (kt + 1) * KT],
                                qT[rb : rb + DH, qs],
                                start=True,
                                stop=True,
                            )
                            ex = epool.tile([128, QB], BF16, tag="ex", name=f"ex{kt}")
                            nc.scalar.activation(ex[:], ps[:], AF.Exp)
                            r = kt - qb * (QB // KT)
                            if r >= 0:
                                # diagonal-crossing tile: zero out k > q
                                nc.vector.tensor_mul(
                                    ex[:], ex[:],
                                    wmask[:, 512 - r * KT : 1024 - r * KT],
                                )
                            exs[kt] = ex
                        # grouped P@V accumulation for this chunk
                        for kt in kts:
                            nc.tensor.matmul(
                                py[: DH + 1],
                                vts[kt][:, 65 * hl : 65 * hl + 65],
                                exs[kt][:],
                                start=(kt == 0),
                                stop=(kt == nkt - 1),
                            )
                    # stash unnormalized y; scatter the denominator row into
                    # this q-block's [2, QB] pair tile (DMA may write any
                    # partition; compute engines may not)
                    denw, dens = den_all[b]
                    p = 2 * qb + hl
                    nc.vector.tensor_copy(
                        denw[:, p * QB : (p + 1) * QB], py[DH : DH + 1, :]
                    )
                    nc.vector.tensor_copy(yT[rb : rb + DH, qs], py[:DH, :])
                    nc.sync.dma_start(
                        dens[qb][hl : hl + 1, :], denw[:, p * QB : (p + 1) * QB]
                    )

                def norm_qb(b, qb):
                    """Reciprocal + normalize for one q-block (both heads)."""
                    _denw, dens = den_all[b]
                    cb = b * T
                    qs = slice(cb + qb * QB, cb + (qb + 1) * QB)
                    recf = dpool.tile([2, QB], F32, tag="recf", name=f"recf{b}_{qb}")
                    nc.vector.reciprocal_approx_fast(recf[:], dens[qb][:])
                    rec = dpool.tile([2, QB], BF16, tag="rec", name=f"rec{b}_{qb}")
                    nc.vector.tensor_copy(rec[:], recf[:])
                    pb = p_psum.tile([128, 512], F32, tag="p", name=f"pb{b}_{qb}")
                    nc.tensor.matmul(
                        pb[:, :QB], esel[:], rec[:],
                        start=True, stop=True,
                    )
                    nc.vector.tensor_mul(yT[:, qs], yT[:, qs], pb[:, :QB])

                def proj_quarter(b, i, deep=False):
                    """Output projection for 4 of the batch's 16 token tiles.

                    deep=True borrows the idle y_psum bank for a 3-deep psum
                    rotation (only safe when no sp unit is accumulating py).
                    """
                    cb = b * T
                    n = 0
                    for tt in range(i * 4, i * 4 + 4):
                        trow = cb + tt * 128
                        for half in range(2):
                            pool = y_psum if (deep and n % 3 == 2) else p_psum
                            tag = "py" if (deep and n % 3 == 2) else "p"
                            pp = pool.tile([128, 512], F32, tag=tag, name=f"pp{b}_{tt}_{half}")
                            n += 1
                            nc.tensor.matmul(
                                pp[:],
                                yT[:, trow : trow + 128],
                                wp_r[:, half * 512 : (half + 1) * 512],
                                start=True,
                                stop=True,
                            )
                            os_ = opool.tile([128, 512], BF16, tag="os", name=f"os{b}_{tt}_{half}")
                            # copy engines 2:1 DVE:ACT — ACT carries the exps
                            if (tt * 2 + half) % 3 == 2:
                                nc.scalar.copy(os_[:], pp[:])
                            else:
                                nc.vector.tensor_copy(os_[:], pp[:])
                            nc.sync.dma_start(
                                out[trow : trow + 128, half * 512 : (half + 1) * 512],
                                os_[:],
                            )

                # ================= emission schedule =================
                # Per batch b: 8 sp units (hl pairs per q-block). After each
                # pair, that q-block is normalized (per-qb reciprocal via a
                # K=2 selector matmul). ALL proj quarters are carried into the
                # next batch's attention stream as fillers, woven 1:1 with the
                # next batch's qkv tiles, so full-power matmul runs never
                # cluster and the serial tail is one quarter. The V transposes
                # for batch b+1 form the (low-power) end block.
                # DMA issue order: the first x tile and wq gate the first
                # matmul, so they go first (issues serialize at ~650ns each).
                dma_x(0)
                nc.sync.dma_start(wq_r[:], wq[:])
                nc.sync.dma_start(wk_r[:], wk[:])
                nc.sync.dma_start(wv_r[:], wv[:])
                nc.sync.dma_start(bqkv_t[:], bqkv[:])
                nc.sync.dma_start(wp_r[:], wp[:])
                nc.sync.dma_start(esel[:], esel_in[:])
                for tt in range(TPB):          # qkv for batch 0
                    qkv_tile(tt)
                for b in range(B):
                    denw = dpool.tile([1, 8 * QB], F32, tag="denw", name=f"denw{b}")
                    dens = [
                        dpool.tile([2, QB], F32, tag="den", name=f"den{b}_{qb}")
                        for qb in range(T // QB)
                    ]
                    den_all[b] = (denw, dens)
                    if b == 0:
                        vts_half(0, 0)
                        vts_half(0, 1)
                    # fillers[i] runs after sp unit i: previous batch's proj
                    # quarters and next batch's qkv tiles, alternating
                    fillers = {}
                    if b == 0:
                        for j, pos in enumerate((2, 3, 5, 6)):
                            fillers[pos] = lambda tt=TPB + j: qkv_tile(tt)
                    elif b < B - 1:
                        fillers = {
                            0: lambda: proj_quarter(b - 1, 0),
                            1: lambda: qkv_tile((b + 1) * TPB + 0),
                            2: lambda: proj_quarter(b - 1, 1),
                            3: lambda: qkv_tile((b + 1) * TPB + 1),
                            4: lambda: proj_quarter(b - 1, 2),
                            5: lambda: qkv_tile((b + 1) * TPB + 2),
                            6: lambda: proj_quarter(b - 1, 3),
                            7: lambda: qkv_tile((b + 1) * TPB + 3),
                        }
                    else:
                        # last batch: weave in b-1's quarters and our own
                        # earlier-normalized quarters
                        fillers = {
                            0: lambda: proj_quarter(b - 1, 0),
                            2: lambda: proj_quarter(b - 1, 1),
                            3: lambda: proj_quarter(b, 0),
                            4: lambda: proj_quarter(b - 1, 2),
                            5: lambda: proj_quarter(b, 1),
                            6: lambda: proj_quarter(b - 1, 3),
                            7: lambda: proj_quarter(b, 2),
                        }
                    sps = [(hl, qb) for qb in range(T // QB) for hl in range(HPC)]
                    for i, (hl, qb) in enumerate(sps):
                        sp_unit(b, hl, qb)
                        if hl == HPC - 1:
                            norm_qb(b, qb)
                        f = fillers.get(i)
                        if f is not None:
                            f()
                    if b + 1 < B:
                        vts_half(b + 1, 0)
                        vts_half(b + 1, 1)
                        vts_all.pop(b, None)
                    else:
                        proj_quarter(b, 3, deep=True)

    nc.compile()
    return nc


def _get_nc():
    global _CACHED_NC
    if _CACHED_NC is None:
        _CACHED_NC = _build()
    return _CACHED_NC


def kernel(x, W_qkv, b_qkv, W_proj, b_proj, _trace=False, _core_ids=None):
    global LAST_RESULT
    x = np.asarray(x, dtype=np.float32)
    W_qkv = np.asarray(W_qkv, dtype=np.float32)
    b_qkv = np.asarray(b_qkv, dtype=np.float32)
    W_proj = np.asarray(W_proj, dtype=np.float32)
    b_proj = np.asarray(b_proj, dtype=np.float32)

    xT = np.ascontiguousarray(x.reshape(NT, C).T).astype(BF16_NP)
    W_qkv_b = W_qkv.astype(BF16_NP)
    W_proj_b = W_proj.astype(BF16_NP)
    esel_np = np.zeros((2, 128), dtype=BF16_NP)
    esel_np[0, :DH] = 1.0
    esel_np[1, DH:] = 1.0
    core_ids = list(range(N_CORES)) if _core_ids is None else _core_ids

    def w_pcn(col0, col1):
        # [C, DPC] -> SBUF layout [p=128, c=CH, n=DPC], contiguous
        w = W_qkv_b[:, col0:col1].reshape(CH, 128, DPC)
        return np.ascontiguousarray(w.transpose(1, 0, 2))

    in_maps = []
    for core in range(len(core_ids)):
        s = slice(core * DPC, (core + 1) * DPC)
        in_maps.append(
            {
                "xT": xT,
                "wq": w_pcn(0 * C + core * DPC, 0 * C + (core + 1) * DPC),
                "wk": w_pcn(1 * C + core * DPC, 1 * C + (core + 1) * DPC),
                "wv": w_pcn(2 * C + core * DPC, 2 * C + (core + 1) * DPC),
                # device computes qT = psq*SCALE + bias, so pre-scale the q bias
                "bqkv": np.ascontiguousarray(
                    np.stack(
                        [
                            b_qkv[0 * C + core * DPC : 0 * C + (core + 1) * DPC] * np.float32(SCALE),
                            b_qkv[1 * C + core * DPC : 1 * C + (core + 1) * DPC],
                            b_qkv[2 * C + core * DPC : 2 * C + (core + 1) * DPC],
                        ],
                        axis=1,
                    )
                ),
                "wp": np.ascontiguousarray(W_proj_b[s, :]),
                "esel": esel_np,
            }
        )

    nc = _get_nc()
    res = run_bass_kernel_spmd(nc, in_maps, core_ids, trace=_trace)
    LAST_RESULT = res

    acc = np.zeros((NT, C), dtype=np.float64)
    for r in res.results:
        acc += r["out"].astype(np.float64)
    acc += b_proj.astype(np.float64)
    return acc.reshape(B, T, C).astype(np.float32)


# revision 37
# speedup vs baseline: 1.1794x; 1.0408x over previous
"""Causal self-attention layer (B=4, T=2048, C=1024, H=16) on 8 TRN2 NeuronCores.

Sharding: Megatron-style tensor parallel over heads — 2 heads per core.
Each core computes q/k/v projections for its 2 heads, causal flash-style
attention with ones-columns on V to accumulate softmax denominators, and a
partial output projection against its 128-row slice of W_proj. The host sums
the 8 partial projections and adds b_proj.

All matmul operands are bfloat16 (pre-cast on host for x/weights; on-chip
activations write bf16 directly out of PSUM). fp32r matmuls run in
fp32_mode=HIGH which power-throttles the PE to 50% for most of the kernel;
bf16 keeps the PE mostly at full rate, halves the x/out DMA bytes, and
doubles DVE throughput on the mask multiplies.

Schedule: the q/k/v projection tiles for batch b+1 are interleaved into the
attention stream of batch b, so the PE never runs a long unbroken stream of
dense projection matmuls (which exhausts the power budget and triggers the
50%-duty throttle), and no phase boundary stalls the PE. V tiles are
transposed once per batch ([128,128] both-heads transpose) with two ones
columns (at free offsets 64 and 129) so each head's 65-wide PV lhsT slice
puts y in psum rows 0-63 and the softmax denominator in row 64. The V
transposes for batch b+1 are emitted in the denominator-reciprocal latency
gap of batch b. PSUM: 4 pools x 2 bufs x 2KB = exactly 8 banks.
"""
import sys

sys.path.insert(0, "/opt/trn_rl_repo")

import numpy as np
import ml_dtypes

import concourse.bass as bass  # noqa: F401
from concourse import bacc
import concourse.mybir as mybir
import concourse.tile as tile
from concourse.bass_utils import run_bass_kernel_spmd
from concourse.masks import make_identity

B, T, C = 4, 2048, 1024
H, DH = 16, 64
N_CORES = 8
HPC = H // N_CORES          # heads per core = 2
DPC = HPC * DH              # head-dims per core = 128
NT = B * T                  # 8192 tokens
CH = C // 128               # 8 contraction chunks
QB = 512                    # q-block width (moving dim)
KT = 128                    # k-tile width (PE partition dim)
CHUNK = 8                   # k-tiles per S/PV emission chunk
SCALE = 1.0 / 8.0           # 1/sqrt(DH)
TPB = T // QB               # qkv token tiles per batch = 4

F32 = mybir.dt.float32
BF16 = mybir.dt.bfloat16
AF = mybir.ActivationFunctionType
BF16_NP = ml_dtypes.bfloat16

_CACHED_NC = None
LAST_RESULT = None


def _build():
    nc = bacc.Bacc(None)

    xT = nc.dram_tensor("xT", [C, NT], BF16, kind="ExternalInput")
    # qkv weights pre-arranged on host to the SBUF layout [p, c, n]
    wq = nc.dram_tensor("wq", [128, CH, DPC], BF16, kind="ExternalInput")
    wk = nc.dram_tensor("wk", [128, CH, DPC], BF16, kind="ExternalInput")
    wv = nc.dram_tensor("wv", [128, CH, DPC], BF16, kind="ExternalInput")
    bqkv = nc.dram_tensor("bqkv", [DPC, 3], F32, kind="ExternalInput")
    wp = nc.dram_tensor("wp", [DPC, C], BF16, kind="ExternalInput")
    # head selector: esel[0,:64]=1, esel[1,64:]=1 — broadcasts a [2,512]
    # reciprocal pair into a [128,512] per-dim normalizer via a K=2 matmul
    esel_in = nc.dram_tensor("esel", [2, 128], BF16, kind="ExternalInput")
    out = nc.dram_tensor("out", [NT, C], BF16, kind="ExternalOutput")

    with tile.TileContext(nc) as tc:
        with (
            tc.tile_pool(name="const", bufs=1) as const,
            tc.tile_pool(name="res", bufs=1) as res,
        ):
            # --- constants (built in f32, cast to bf16 once) ---
            ident = const.tile([128, 128], BF16, tag="ident")
            # sliding causal mask: wmask[k, u] = 1 iff k <= u - 512; a crossing
            # tile r multiplies by wmask[:, 512-128r : 1024-128r]
            wmask = const.tile([128, 1024], BF16, tag="wmask")
            ones_col = const.tile([128, 1], BF16, tag="ones_col")
            esel = const.tile([2, 128], BF16, tag="esel")
            with tc.tile_pool(name="cstage", bufs=1) as cstage:
                ident_s = cstage.tile([128, 128], F32, tag="ident_s")
                make_identity(nc, ident_s[:])
                nc.vector.tensor_copy(ident[:], ident_s[:])

                wmask_s = cstage.tile([128, 1024], F32, tag="wmask_s")
                nc.gpsimd.memset(wmask_s[:], 0.0)
                nc.gpsimd.affine_select(
                    out=wmask_s[:],
                    in_=wmask_s[:],
                    compare_op=mybir.AluOpType.is_gt,
                    fill=1.0,
                    base=512,
                    # keep 0 where (512 + k - u) > 0, fill 1 where k <= u - 512
                    pattern=[[-1, 1024]],
                    channel_multiplier=1,
                )
                nc.vector.tensor_copy(wmask[:], wmask_s[:])

                ones_s = cstage.tile([128, 1], F32, tag="ones_s")
                nc.gpsimd.memset(ones_s[:], 1.0)
                nc.vector.tensor_copy(ones_col[:], ones_s[:])

            bqkv_t = const.tile([DPC, 3], F32, tag="bqkv")
            bq_t, bk_t, bv_t = bqkv_t[:, 0:1], bqkv_t[:, 1:2], bqkv_t[:, 2:3]

            # weights -> SBUF directly in bf16 (cast on host)
            wq_r = const.tile([128, CH, DPC], BF16, tag="wq_r")
            wk_r = const.tile([128, CH, DPC], BF16, tag="wk_r")
            wv_r = const.tile([128, CH, DPC], BF16, tag="wv_r")
            wp_r = const.tile([DPC, C], BF16, tag="wp_r")

            # --- residents ---
            qT = res.tile([DPC, NT], BF16, tag="qT")
            kT = res.tile([DPC, NT], BF16, tag="kT")
            vT = res.tile([DPC, NT], BF16, tag="vT")
            yT = res.tile([DPC, NT], BF16, tag="yT")

            xT_re = xT.rearrange("(c p) t -> p c t", p=128)
            n_ktiles = T // KT  # 16

            with (
                tc.tile_pool(name="xpool", bufs=3) as xpool,
                tc.tile_pool(name="vpool", bufs=34) as vpool,
                tc.tile_pool(name="epool", bufs=CHUNK + 3) as epool,
                tc.tile_pool(name="dpool", bufs=2) as dpool,
                tc.tile_pool(name="opool", bufs=8) as opool,
                tc.tile_pool(name="q_psum", bufs=2, space="PSUM") as q_psum,
                tc.tile_pool(name="s_psum", bufs=3, space="PSUM") as s_psum,
                tc.tile_pool(name="y_psum", bufs=1, space="PSUM") as y_psum,
                tc.tile_pool(name="p_psum", bufs=2, space="PSUM") as p_psum,
            ):
                xs_tiles = {}

                def dma_x(tt):
                    if tt >= NT // QB or tt in xs_tiles:
                        return
                    xs = xpool.tile([128, CH, QB], BF16, tag="xs", name=f"xs{tt}")
                    if tt == 0:
                        # split the first tile across 4 DMA queues — it gates
                        # the very first matmul (a single queue moves ~110GB/s)
                        for c0 in range(0, CH, 2):
                            nc.sync.dma_start(
                                xs[:, c0 : c0 + 2, :],
                                xT_re[:, c0 : c0 + 2, tt * QB : (tt + 1) * QB],
                            )
                    else:
                        nc.sync.dma_start(
                            xs[:], xT_re[:, :, tt * QB : (tt + 1) * QB]
                        )
                    xs_tiles[tt] = xs

                def qkv_tile(tt):
                    """Project one 512-token tile into qT/kT/vT; prefetch x."""
                    dma_x(tt + 1)
                    xs = xs_tiles.pop(tt)
                    ts_ = slice(tt * QB, (tt + 1) * QB)
                    psq = q_psum.tile([128, QB], F32, tag="qkv", name=f"psq{tt}")
                    psk = q_psum.tile([128, QB], F32, tag="qkv", name=f"psk{tt}")
                    psv = q_psum.tile([128, QB], F32, tag="qkv", name=f"psv{tt}")
                    for ps, w_r in ((psq, wq_r), (psk, wk_r), (psv, wv_r)):
                        for c in range(CH):
                            nc.tensor.matmul(
                                ps[:], w_r[:, c, :], xs[:, c, :],
                                start=(c == 0), stop=(c == CH - 1),
                            )
                    # copy out of PSUM (+bias; q also scaled by 1/sqrt(dh))
                    nc.scalar.activation(qT[:, ts_], psq[:], AF.Identity, bias=bq_t[:], scale=SCALE)
                    nc.vector.tensor_scalar_add(kT[:, ts_], psk[:], bk_t[:])
                    nc.vector.tensor_scalar_add(vT[:, ts_], psv[:], bv_t[:])

                # per-batch state
                vts_all = {}   # b -> list of 16 [128, 130] tiles
                den_all = {}   # b -> (denw, den)

                def vts_half(b, half):
                    """Transpose 8 V token-tiles (both heads at once).

                    v tile layout [128 tok, 130]: cols 0-63 head0 dims, col 64
                    ones, cols 65-128 head1 dims, col 129 ones. Head hl's PV
                    lhsT is v[:, 65*hl : 65*hl+65] -> psum rows 0-63 = y,
                    row 64 = denominator.
                    """
                    cb = b * T
                    vts = vts_all.setdefault(b, [None] * n_ktiles)
                    for kt in range(half * 8, half * 8 + 8):
                        pt = s_psum.tile([128, 512], BF16, tag="s", name=f"pt{b}_{kt}")
                        nc.tensor.transpose(
                            pt[:, :128],
                            vT[:, cb + kt * KT : cb + (kt + 1) * KT],
                            ident[:],
                        )
                        v = vpool.tile([128, 130], BF16, tag="v", name=f"v{b}_{kt}")
                        nc.vector.tensor_copy(v[:, 0:64], pt[:, 0:64])
                        nc.scalar.copy(v[:, 65:129], pt[:, 64:128])
                        nc.vector.tensor_copy(v[:, 64:65], ones_col[:])
                        nc.vector.tensor_copy(v[:, 129:130], ones_col[:])
                        vts[kt] = v

                def sp_unit(b, hl, qb):
                    """Scores + exp + P@V for one (head, q-block)."""
                    cb = b * T
                    rb = hl * DH
                    vts = vts_all[b]
                    qs = slice(cb + qb * QB, cb + (qb + 1) * QB)
                    py = y_psum.tile([128, QB], F32, tag="py", name=f"py{b}_{hl}_{qb}")
                    nkt = (qb + 1) * (QB // KT)
                    for k0 in range(0, nkt, CHUNK):
                        kts = range(k0, min(k0 + CHUNK, nkt))
                        exs = {}
                        # scores + exp for this chunk
                        for kt in kts:
                            ps = s_psum.tile([128, QB], F32, tag="s", name=f"ps{kt}")
                            nc.tensor.matmul(
                                ps[:],
                                kT[rb : rb + DH, cb + kt * KT : cb + (kt + 1) * KT],
                                qT[rb : rb + DH, qs],
                                start=True,
                                stop=True,
                            )
                            ex = epool.tile([128, QB], BF16, tag="ex", name=f"ex{kt}")
                            nc.scalar.activation(ex[:], ps[:], AF.Exp)
                            r = kt - qb * (QB // KT)
                            if r >= 0:
                                # diagonal-crossing tile: zero out k > q
                                nc.vector.tensor_mul(
                                    ex[:], ex[:],
                                    wmask[:, 512 - r * KT : 1024 - r * KT],
                                )
                            exs[kt] = ex
                        # grouped P@V accumulation for this chunk
                        for kt in kts:
                            nc.tensor.matmul(
                                py[: DH + 1],
                                vts[kt][:, 65 * hl : 65 * hl + 65],
                                exs[kt][:],
                                start=(kt == 0),
                                stop=(kt == nkt - 1),
                            )
                    # stash unnormalized y; scatter the denominator row into
                    # this q-block's [2, QB] pair tile (DMA may write any
                    # partition; compute engines may not)
                    denw, dens = den_all[b]
                    p = 2 * qb + hl
                    nc.vector.tensor_copy(
                        denw[:, p * QB : (p + 1) * QB], py[DH : DH + 1, :]
                    )
                    nc.vector.tensor_copy(yT[rb : rb + DH, qs], py[:DH, :])
                    nc.sync.dma_start(
                        dens[qb][hl : hl + 1, :], denw[:, p * QB : (p + 1) * QB]
                    )

                def norm_qb(b, qb):
                    """Reciprocal + normalize for one q-block (both heads)."""
                    _denw, dens = den_all[b]
                    cb = b * T
                    qs = slice(cb + qb * QB, cb + (qb + 1) * QB)
                    recf = dpool.tile([2, QB], F32, tag="recf", name=f"recf{b}_{qb}")
                    nc.vector.reciprocal_approx_fast(recf[:], dens[qb][:])
                    rec = dpool.tile([2, QB], BF16, tag="rec", name=f"rec{b}_{qb}")
                    nc.vector.tensor_copy(rec[:], recf[:])
                    pb = p_psum.tile([128, 512], F32, tag="p", name=f"pb{b}_{qb}")
                    nc.tensor.matmul(
                        pb[:, :QB], esel[:], rec[:],
                        start=True, stop=True,
                    )
                    nc.vector.tensor_mul(yT[:, qs], yT[:, qs], pb[:, :QB])

                def proj_quarter(b, i, deep=False):
                    """Output projection for 4 of the batch's 16 token tiles.

                    deep=True borrows the idle y_psum bank for a 3-deep psum
                    rotation (only safe when no sp unit is accumulating py).
                    """
                    cb = b * T
                    n = 0
                    for tt in range(i * 4, i * 4 + 4):
                        trow = cb + tt * 128
                        for half in range(2):
                            pool = y_psum if (deep and n % 3 == 2) else p_psum
                            tag = "py" if (deep and n % 3 == 2) else "p"
                            pp = pool.tile([128, 512], F32, tag=tag, name=f"pp{b}_{tt}_{half}")
                            n += 1
                            nc.tensor.matmul(
                                pp[:],
                                yT[:, trow : trow + 128],
                                wp_r[:, half * 512 : (half + 1) * 512],
                                start=True,
                                stop=True,
                            )
                            os_ = opool.tile([128, 512], BF16, tag="os", name=f"os{b}_{tt}_{half}")
                            # copy engines 2:1 DVE:ACT — ACT carries the exps
                            if (tt * 2 + half) % 3 == 2:
                                nc.scalar.copy(os_[:], pp[:])
                            else:
                                nc.vector.tensor_copy(os_[:], pp[:])
                            nc.sync.dma_start(
                                out[trow : trow + 128, half * 512 : (half + 1) * 512],
                                os_[:],
                            )

                # ================= emission schedule =================
                # Per batch b: 8 sp units (hl pairs per q-block). After each
                # pair, that q-block is normalized (per-qb reciprocal via a
                # K=2 selector matmul). ALL proj quarters are carried into the
                # next batch's attention stream as fillers, woven 1:1 with the
                # next batch's qkv tiles, so full-power matmul runs never
                # cluster and the serial tail is one quarter. The V transposes
                # for batch b+1 form the (low-power) end block.
                # DMA issue order: the first x tile and wq gate the first
                # matmul, so they go first (issues serialize at ~650ns each).
                dma_x(0)
                nc.sync.dma_start(wq_r[:], wq[:])
                nc.sync.dma_start(wk_r[:], wk[:])
                nc.sync.dma_start(wv_r[:], wv[:])
                nc.sync.dma_start(bqkv_t[:], bqkv[:])
                nc.sync.dma_start(wp_r[:], wp[:])
                nc.sync.dma_start(esel[:], esel_in[:])
                # qkv for batch 0, with its V transposes woven in (each vts
                # half only needs the first two / all four tiles' vT)
                qkv_tile(0)
                qkv_tile(1)
                vts_half(0, 0)
                qkv_tile(2)
                qkv_tile(3)
                vts_half(0, 1)
                for b in range(B):
                    denw = dpool.tile([1, 8 * QB], F32, tag="denw", name=f"denw{b}")
                    dens = [
                        dpool.tile([2, QB], F32, tag="den", name=f"den{b}_{qb}")
                        for qb in range(T // QB)
                    ]
                    den_all[b] = (denw, dens)
                    # fillers[i] runs after sp unit i: previous batch's proj
                    # quarters and next batch's qkv tiles, alternating
                    fillers = {}
                    if b == 0:
                        for j, pos in enumerate((2, 3, 5, 6)):
                            fillers[pos] = lambda tt=TPB + j: qkv_tile(tt)
                    elif b < B - 1:
                        fillers = {
                            0: lambda: proj_quarter(b - 1, 0),
                            1: lambda: qkv_tile((b + 1) * TPB + 0),
                            2: lambda: proj_quarter(b - 1, 1),
                            3: lambda: qkv_tile((b + 1) * TPB + 1),
                            4: lambda: proj_quarter(b - 1, 2),
                            5: lambda: qkv_tile((b + 1) * TPB + 2),
                            6: lambda: proj_quarter(b - 1, 3),
                            7: lambda: qkv_tile((b + 1) * TPB + 3),
                        }
                    else:
                        # last batch: weave in b-1's quarters and our own
                        # earlier-normalized quarters
                        fillers = {
                            0: lambda: proj_quarter(b - 1, 0),
                            2: lambda: proj_quarter(b - 1, 1),
                            3: lambda: proj_quarter(b, 0),
                            4: lambda: proj_quarter(b - 1, 2),
                            5: lambda: proj_quarter(b, 1),
                            6: lambda: proj_quarter(b - 1, 3),
                            7: lambda: proj_quarter(b, 2),
                        }
                    sps = [(hl, qb) for qb in range(T // QB) for hl in range(HPC)]
                    for i, (hl, qb) in enumerate(sps):
                        sp_unit(b, hl, qb)
                        if hl == HPC - 1:
                            norm_qb(b, qb)
                        f = fillers.get(i)
                        if f is not None:
                            f()
                    if b + 1 < B:
                        vts_half(b + 1, 0)
                        vts_half(b + 1, 1)
                        vts_all.pop(b, None)
                    else:
                        proj_quarter(b, 3, deep=True)

    nc.compile()
    return nc


def _get_nc():
    global _CACHED_NC
    if _CACHED_NC is None:
        _CACHED_NC = _build()
    return _CACHED_NC


def kernel(x, W_qkv, b_qkv, W_proj, b_proj, _trace=False, _core_ids=None):
    global LAST_RESULT
    x = np.asarray(x, dtype=np.float32)
    W_qkv = np.asarray(W_qkv, dtype=np.float32)
    b_qkv = np.asarray(b_qkv, dtype=np.float32)
    W_proj = np.asarray(W_proj, dtype=np.float32)
    b_proj = np.asarray(b_proj, dtype=np.float32)

    xT = np.ascontiguousarray(x.reshape(NT, C).T).astype(BF16_NP)
    W_qkv_b = W_qkv.astype(BF16_NP)
    W_proj_b = W_proj.astype(BF16_NP)
    esel_np = np.zeros((2, 128), dtype=BF16_NP)
    esel_np[0, :DH] = 1.0
    esel_np[1, DH:] = 1.0
    core_ids = list(range(N_CORES)) if _core_ids is None else _core_ids

    def w_pcn(col0, col1):
        # [C, DPC] -> SBUF layout [p=128, c=CH, n=DPC], contiguous
        w = W_qkv_b[:, col0:col1].reshape(CH, 128, DPC)
        return np.ascontiguousarray(w.transpose(1, 0, 2))

    in_maps = []
    for core in range(len(core_ids)):
        s = slice(core * DPC, (core + 1) * DPC)
        in_maps.append(
            {
                "xT": xT,
                "wq": w_pcn(0 * C + core * DPC, 0 * C + (core + 1) * DPC),
                "wk": w_pcn(1 * C + core * DPC, 1 * C + (core + 1) * DPC),
                "wv": w_pcn(2 * C + core * DPC, 2 * C + (core + 1) * DPC),
                # device computes qT = psq*SCALE + bias, so pre-scale the q bias
                "bqkv": np.ascontiguousarray(
                    np.stack(
                        [
                            b_qkv[0 * C + core * DPC : 0 * C + (core + 1) * DPC] * np.float32(SCALE),
                            b_qkv[1 * C + core * DPC : 1 * C + (core + 1) * DPC],
                            b_qkv[2 * C + core * DPC : 2 * C + (core + 1) * DPC],
                        ],
                        axis=1,
                    )
                ),
                "wp": np.ascontiguousarray(W_proj_b[s, :]),
                "esel": esel_np,
            }
        )

    nc = _get_nc()
    res = run_bass_kernel_spmd(nc, in_maps, core_ids, trace=_trace)
    LAST_RESULT = res

    acc = np.zeros((NT, C), dtype=np.float64)
    for r in res.results:
        acc += r["out"].astype(np.float64)
    acc += b_proj.astype(np.float64)
    return acc.reshape(B, T, C).astype(np.float32)


# revision 41
# speedup vs baseline: 1.2108x; 1.0266x over previous
"""Causal self-attention layer (B=4, T=2048, C=1024, H=16) on 8 TRN2 NeuronCores.

Sharding: Megatron-style tensor parallel over heads — 2 heads per core.
Each core computes q/k/v projections for its 2 heads, causal flash-style
attention with ones-columns on V to accumulate softmax denominators, and a
partial output projection against its 128-row slice of W_proj. The host sums
the 8 partial projections and adds b_proj.

All matmul operands are bfloat16 (pre-cast on host for x/weights; on-chip
activations write bf16 directly out of PSUM). fp32r matmuls run in
fp32_mode=HIGH which power-throttles the PE to 50% for most of the kernel;
bf16 keeps the PE mostly at full rate, halves the x/out DMA bytes, and
doubles DVE throughput on the mask multiplies.

Schedule: the q/k/v projection tiles for batch b+1 are interleaved into the
attention stream of batch b, so the PE never runs a long unbroken stream of
dense projection matmuls (which exhausts the power budget and triggers the
50%-duty throttle), and no phase boundary stalls the PE. V tiles are
transposed once per batch ([128,128] both-heads transpose) with two ones
columns (at free offsets 64 and 129) so each head's 65-wide PV lhsT slice
puts y in psum rows 0-63 and the softmax denominator in row 64. The V
transposes for batch b+1 are emitted in the denominator-reciprocal latency
gap of batch b. PSUM: 4 pools x 2 bufs x 2KB = exactly 8 banks.
"""
import sys

sys.path.insert(0, "/opt/trn_rl_repo")

import numpy as np
import ml_dtypes

import concourse.bass as bass  # noqa: F401
from concourse import bacc
import concourse.mybir as mybir
import concourse.tile as tile
from concourse.bass_utils import run_bass_kernel_spmd
from concourse.masks import make_identity

B, T, C = 4, 2048, 1024
H, DH = 16, 64
N_CORES = 8
HPC = H // N_CORES          # heads per core = 2
DPC = HPC * DH              # head-dims per core = 128
NT = B * T                  # 8192 tokens
CH = C // 128               # 8 contraction chunks
QB = 512                    # q-block width (moving dim)
KT = 128                    # k-tile width (PE partition dim)
CHUNK = 8                   # k-tiles per S/PV emission chunk
SCALE = 1.0 / 8.0           # 1/sqrt(DH)
TPB = T // QB               # qkv token tiles per batch = 4

F32 = mybir.dt.float32
BF16 = mybir.dt.bfloat16
AF = mybir.ActivationFunctionType
BF16_NP = ml_dtypes.bfloat16

_CACHED_NC = None
LAST_RESULT = None


def _build():
    nc = bacc.Bacc(None)

    xT = nc.dram_tensor("xT", [C, NT], BF16, kind="ExternalInput")
    # qkv weights pre-arranged on host to the SBUF layout [p, c, n]
    wq = nc.dram_tensor("wq", [128, CH, DPC], BF16, kind="ExternalInput")
    wk = nc.dram_tensor("wk", [128, CH, DPC], BF16, kind="ExternalInput")
    wv = nc.dram_tensor("wv", [128, CH, DPC], BF16, kind="ExternalInput")
    bqkv = nc.dram_tensor("bqkv", [DPC, 3], F32, kind="ExternalInput")
    wp = nc.dram_tensor("wp", [DPC, C], BF16, kind="ExternalInput")
    # head selector: esel[0,:64]=1, esel[1,64:]=1 — broadcasts a [2,512]
    # reciprocal pair into a [128,512] per-dim normalizer via a K=2 matmul
    esel_in = nc.dram_tensor("esel", [2, 128], BF16, kind="ExternalInput")
    out = nc.dram_tensor("out", [NT, C], BF16, kind="ExternalOutput")

    with tile.TileContext(nc) as tc:
        with (
            tc.tile_pool(name="const", bufs=1) as const,
            tc.tile_pool(name="res", bufs=1) as res,
        ):
            # --- constants (built in f32, cast to bf16 once) ---
            ident = const.tile([128, 128], BF16, tag="ident")
            # sliding causal mask: wmask[k, u] = 1 iff k <= u - 512; a crossing
            # tile r multiplies by wmask[:, 512-128r : 1024-128r]
            wmask = const.tile([128, 1024], BF16, tag="wmask")
            ones_col = const.tile([128, 1], BF16, tag="ones_col")
            esel = const.tile([2, 128], BF16, tag="esel")
            with tc.tile_pool(name="cstage", bufs=1) as cstage:
                ident_s = cstage.tile([128, 128], F32, tag="ident_s")
                make_identity(nc, ident_s[:])
                nc.vector.tensor_copy(ident[:], ident_s[:])

                wmask_s = cstage.tile([128, 1024], F32, tag="wmask_s")
                nc.gpsimd.memset(wmask_s[:], 0.0)
                nc.gpsimd.affine_select(
                    out=wmask_s[:],
                    in_=wmask_s[:],
                    compare_op=mybir.AluOpType.is_gt,
                    fill=1.0,
                    base=512,
                    # keep 0 where (512 + k - u) > 0, fill 1 where k <= u - 512
                    pattern=[[-1, 1024]],
                    channel_multiplier=1,
                )
                nc.vector.tensor_copy(wmask[:], wmask_s[:])

                ones_s = cstage.tile([128, 1], F32, tag="ones_s")
                nc.gpsimd.memset(ones_s[:], 1.0)
                nc.vector.tensor_copy(ones_col[:], ones_s[:])

            bqkv_t = const.tile([DPC, 3], F32, tag="bqkv")
            bq_t, bk_t, bv_t = bqkv_t[:, 0:1], bqkv_t[:, 1:2], bqkv_t[:, 2:3]

            # weights -> SBUF directly in bf16 (cast on host)
            wq_r = const.tile([128, CH, DPC], BF16, tag="wq_r")
            wk_r = const.tile([128, CH, DPC], BF16, tag="wk_r")
            wv_r = const.tile([128, CH, DPC], BF16, tag="wv_r")
            wp_r = const.tile([DPC, C], BF16, tag="wp_r")

            # --- residents ---
            qT = res.tile([DPC, NT], BF16, tag="qT")
            kT = res.tile([DPC, NT], BF16, tag="kT")
            vT = res.tile([DPC, NT], BF16, tag="vT")
            yT = res.tile([DPC, NT], BF16, tag="yT")

            xT_re = xT.rearrange("(c p) t -> p c t", p=128)
            n_ktiles = T // KT  # 16

            with (
                tc.tile_pool(name="xpool", bufs=3) as xpool,
                tc.tile_pool(name="vpool", bufs=34) as vpool,
                tc.tile_pool(name="epool", bufs=CHUNK + 3) as epool,
                tc.tile_pool(name="dpool", bufs=2) as dpool,
                tc.tile_pool(name="opool", bufs=8) as opool,
                tc.tile_pool(name="q_psum", bufs=2, space="PSUM") as q_psum,
                tc.tile_pool(name="s_psum", bufs=3, space="PSUM") as s_psum,
                tc.tile_pool(name="y_psum", bufs=1, space="PSUM") as y_psum,
                tc.tile_pool(name="p_psum", bufs=2, space="PSUM") as p_psum,
            ):
                xs_tiles = {}

                def dma_x(tt):
                    if tt >= NT // QB or tt in xs_tiles:
                        return
                    xs = xpool.tile([128, CH, QB], BF16, tag="xs", name=f"xs{tt}")
                    if tt == 0:
                        # split the first tile across 4 DMA queues — it gates
                        # the very first matmul (a single queue moves ~110GB/s)
                        for c0 in range(0, CH, 2):
                            nc.sync.dma_start(
                                xs[:, c0 : c0 + 2, :],
                                xT_re[:, c0 : c0 + 2, tt * QB : (tt + 1) * QB],
                            )
                    else:
                        nc.sync.dma_start(
                            xs[:], xT_re[:, :, tt * QB : (tt + 1) * QB]
                        )
                    xs_tiles[tt] = xs

                def qkv_tile(tt):
                    """Project one 512-token tile into qT/kT/vT; prefetch x."""
                    dma_x(tt + 1)
                    xs = xs_tiles.pop(tt)
                    ts_ = slice(tt * QB, (tt + 1) * QB)
                    psq = q_psum.tile([128, QB], F32, tag="qkv", name=f"psq{tt}")
                    psk = q_psum.tile([128, QB], F32, tag="qkv", name=f"psk{tt}")
                    psv = q_psum.tile([128, QB], F32, tag="qkv", name=f"psv{tt}")
                    for ps, w_r in ((psq, wq_r), (psk, wk_r), (psv, wv_r)):
                        for c in range(CH):
                            nc.tensor.matmul(
                                ps[:], w_r[:, c, :], xs[:, c, :],
                                start=(c == 0), stop=(c == CH - 1),
                            )
                    # copy out of PSUM (+bias; q also scaled by 1/sqrt(dh))
                    nc.scalar.activation(qT[:, ts_], psq[:], AF.Identity, bias=bq_t[:], scale=SCALE)
                    nc.vector.tensor_scalar_add(kT[:, ts_], psk[:], bk_t[:])
                    nc.vector.tensor_scalar_add(vT[:, ts_], psv[:], bv_t[:])

                # per-batch state
                vts_all = {}   # b -> list of 16 [128, 130] tiles
                den_all = {}   # b -> (denw, den)

                def vts_half(b, half):
                    """Transpose 8 V token-tiles (both heads at once).

                    v tile layout [128 tok, 130]: cols 0-63 head0 dims, col 64
                    ones, cols 65-128 head1 dims, col 129 ones. Head hl's PV
                    lhsT is v[:, 65*hl : 65*hl+65] -> psum rows 0-63 = y,
                    row 64 = denominator.
                    """
                    cb = b * T
                    vts = vts_all.setdefault(b, [None] * n_ktiles)
                    for kt in range(half * 8, half * 8 + 8):
                        pt = s_psum.tile([128, 512], BF16, tag="s", name=f"pt{b}_{kt}")
                        nc.tensor.transpose(
                            pt[:, :128],
                            vT[:, cb + kt * KT : cb + (kt + 1) * KT],
                            ident[:],
                        )
                        v = vpool.tile([128, 130], BF16, tag="v", name=f"v{b}_{kt}")
                        nc.vector.tensor_copy(v[:, 0:64], pt[:, 0:64])
                        nc.scalar.copy(v[:, 65:129], pt[:, 64:128])
                        nc.vector.tensor_copy(v[:, 64:65], ones_col[:])
                        nc.vector.tensor_copy(v[:, 129:130], ones_col[:])
                        vts[kt] = v

                def sp_unit(b, hl, qb):
                    """Scores + exp + P@V for one (head, q-block)."""
                    cb = b * T
                    rb = hl * DH
                    vts = vts_all[b]
                    qs = slice(cb + qb * QB, cb + (qb + 1) * QB)
                    py = y_psum.tile([128, QB], F32, tag="py", name=f"py{b}_{hl}_{qb}")
                    nkt = (qb + 1) * (QB // KT)
                    for k0 in range(0, nkt, CHUNK):
                        kts = range(k0, min(k0 + CHUNK, nkt))
                        exs = {}
                        # scores + exp for this chunk
                        for kt in kts:
                            ps = s_psum.tile([128, QB], F32, tag="s", name=f"ps{kt}")
                            nc.tensor.matmul(
                                ps[:],
                                kT[rb : rb + DH, cb + kt * KT : cb + (kt + 1) * KT],
                                qT[rb : rb + DH, qs],
                                start=True,
                                stop=True,
                            )
                            ex = epool.tile([128, QB], BF16, tag="ex", name=f"ex{kt}")
                            nc.scalar.activation(ex[:], ps[:], AF.Exp)
                            r = kt - qb * (QB // KT)
                            if r >= 0:
                                # diagonal-crossing tile: zero out k > q
                                nc.vector.tensor_mul(
                                    ex[:], ex[:],
                                    wmask[:, 512 - r * KT : 1024 - r * KT],
                                )
                            exs[kt] = ex
                        # grouped P@V accumulation for this chunk
                        for kt in kts:
                            nc.tensor.matmul(
                                py[: DH + 1],
                                vts[kt][:, 65 * hl : 65 * hl + 65],
                                exs[kt][:],
                                start=(kt == 0),
                                stop=(kt == nkt - 1),
                            )
                    # stash unnormalized y; scatter the denominator row into
                    # this q-block's [2, QB] pair tile (DMA may write any
                    # partition; compute engines may not)
                    denw, dens = den_all[b]
                    p = 2 * qb + hl
                    nc.vector.tensor_copy(
                        denw[:, p * QB : (p + 1) * QB], py[DH : DH + 1, :]
                    )
                    nc.vector.tensor_copy(yT[rb : rb + DH, qs], py[:DH, :])
                    nc.sync.dma_start(
                        dens[qb][hl : hl + 1, :], denw[:, p * QB : (p + 1) * QB]
                    )

                def norm_qb(b, qb):
                    """Reciprocal + normalize for one q-block (both heads)."""
                    _denw, dens = den_all[b]
                    cb = b * T
                    qs = slice(cb + qb * QB, cb + (qb + 1) * QB)
                    recf = dpool.tile([2, QB], F32, tag="recf", name=f"recf{b}_{qb}")
                    nc.vector.reciprocal_approx_fast(recf[:], dens[qb][:])
                    rec = dpool.tile([2, QB], BF16, tag="rec", name=f"rec{b}_{qb}")
                    if b == B - 1 and qb >= 2:
                        # endgame: no exps left, ACT is idle while DVE drains
                        nc.scalar.copy(rec[:], recf[:])
                    else:
                        nc.vector.tensor_copy(rec[:], recf[:])
                    pb = p_psum.tile([128, 512], F32, tag="p", name=f"pb{b}_{qb}")
                    nc.tensor.matmul(
                        pb[:, :QB], esel[:], rec[:],
                        start=True, stop=True,
                    )
                    nc.vector.tensor_mul(yT[:, qs], yT[:, qs], pb[:, :QB])

                def proj_quarter(b, i, deep=False):
                    """Output projection for 4 of the batch's 16 token tiles.

                    deep=True borrows the idle y_psum bank for a 3-deep psum
                    rotation (only safe when no sp unit is accumulating py).
                    """
                    act_heavy = b == B - 1 and i >= 2
                    cb = b * T
                    n = 0
                    for tt in range(i * 4, i * 4 + 4):
                        trow = cb + tt * 128
                        for half in range(2):
                            pool = y_psum if (deep and n % 3 == 2) else p_psum
                            tag = "py" if (deep and n % 3 == 2) else "p"
                            pp = pool.tile([128, 512], F32, tag=tag, name=f"pp{b}_{tt}_{half}")
                            n += 1
                            nc.tensor.matmul(
                                pp[:],
                                yT[:, trow : trow + 128],
                                wp_r[:, half * 512 : (half + 1) * 512],
                                start=True,
                                stop=True,
                            )
                            os_ = opool.tile([128, 512], BF16, tag="os", name=f"os{b}_{tt}_{half}")
                            # copy engines 2:1 DVE:ACT — ACT carries the exps;
                            # flipped in the endgame where ACT is idle
                            on_act = (tt * 2 + half) % 3 == 2
                            if act_heavy:
                                on_act = not on_act
                            if on_act:
                                nc.scalar.copy(os_[:], pp[:])
                            else:
                                nc.vector.tensor_copy(os_[:], pp[:])
                            nc.sync.dma_start(
                                out[trow : trow + 128, half * 512 : (half + 1) * 512],
                                os_[:],
                            )

                # ================= emission schedule =================
                # Per batch b: 8 sp units (hl pairs per q-block). After each
                # pair, that q-block is normalized (per-qb reciprocal via a
                # K=2 selector matmul). ALL proj quarters are carried into the
                # next batch's attention stream as fillers, woven 1:1 with the
                # next batch's qkv tiles, so full-power matmul runs never
                # cluster and the serial tail is one quarter. The V transposes
                # for batch b+1 form the (low-power) end block.
                # DMA issue order: the first x tile and wq gate the first
                # matmul, so they go first (issues serialize at ~650ns each).
                dma_x(0)
                nc.sync.dma_start(wq_r[:], wq[:])
                nc.sync.dma_start(wk_r[:], wk[:])
                nc.sync.dma_start(wv_r[:], wv[:])
                nc.sync.dma_start(bqkv_t[:], bqkv[:])
                nc.sync.dma_start(wp_r[:], wp[:])
                nc.sync.dma_start(esel[:], esel_in[:])
                # qkv for batch 0, with its V transposes woven in (each vts
                # half only needs the first two / all four tiles' vT)
                qkv_tile(0)
                qkv_tile(1)
                vts_half(0, 0)
                qkv_tile(2)
                qkv_tile(3)
                vts_half(0, 1)
                for b in range(B):
                    denw = dpool.tile([1, 8 * QB], F32, tag="denw", name=f"denw{b}")
                    dens = [
                        dpool.tile([2, QB], F32, tag="den", name=f"den{b}_{qb}")
                        for qb in range(T // QB)
                    ]
                    den_all[b] = (denw, dens)
                    # fillers[i] runs after sp unit i: previous batch's proj
                    # quarters and next batch's qkv tiles, alternating
                    fillers = {}
                    if b == 0:
                        for j, pos in enumerate((2, 3, 5, 6)):
                            fillers[pos] = lambda tt=TPB + j: qkv_tile(tt)
                    elif b < B - 1:
                        fillers = {
                            0: lambda: proj_quarter(b - 1, 0),
                            1: lambda: qkv_tile((b + 1) * TPB + 0),
                            2: lambda: proj_quarter(b - 1, 1),
                            3: lambda: qkv_tile((b + 1) * TPB + 1),
                            4: lambda: proj_quarter(b - 1, 2),
                            5: lambda: qkv_tile((b + 1) * TPB + 2),
                            6: lambda: proj_quarter(b - 1, 3),
                            7: lambda: qkv_tile((b + 1) * TPB + 3),
                        }
                    else:
                        # last batch: weave in b-1's quarters and our own
                        # earlier-normalized quarters
                        fillers = {
                            0: lambda: proj_quarter(b - 1, 0),
                            2: lambda: proj_quarter(b - 1, 1),
                            3: lambda: proj_quarter(b, 0),
                            4: lambda: proj_quarter(b - 1, 2),
                            5: lambda: proj_quarter(b, 1),
                            6: lambda: proj_quarter(b - 1, 3),
                            7: lambda: proj_quarter(b, 2),
                        }
                    sps = [(hl, qb) for qb in range(T // QB) for hl in range(HPC)]
                    for i, (hl, qb) in enumerate(sps):
                        sp_unit(b, hl, qb)
                        # filler first: its matmuls cover the reciprocal
                        # chain's cross-engine latency before the pb matmul
                        f = fillers.get(i)
                        if f is not None:
                            f()
                        if hl == HPC - 1:
                            norm_qb(b, qb)
                    if b + 1 < B:
                        vts_half(b + 1, 0)
                        vts_half(b + 1, 1)
                        vts_all.pop(b, None)
                    else:
                        proj_quarter(b, 3, deep=True)

    nc.compile()
    return nc


def _get_nc():
    global _CACHED_NC
    if _CACHED_NC is None:
        _CACHED_NC = _build()
    return _CACHED_NC


def kernel(x, W_qkv, b_qkv, W_proj, b_proj, _trace=False, _core_ids=None):
    global LAST_RESULT
    x = np.asarray(x, dtype=np.float32)
    W_qkv = np.asarray(W_qkv, dtype=np.float32)
    b_qkv = np.asarray(b_qkv, dtype=np.float32)
    W_proj = np.asarray(W_proj, dtype=np.float32)
    b_proj = np.asarray(b_proj, dtype=np.float32)

    xT = np.ascontiguousarray(x.reshape(NT, C).T).astype(BF16_NP)
    W_qkv_b = W_qkv.astype(BF16_NP)
    W_proj_b = W_proj.astype(BF16_NP)
    esel_np = np.zeros((2, 128), dtype=BF16_NP)
    esel_np[0, :DH] = 1.0
    esel_np[1, DH:] = 1.0
    core_ids = list(range(N_CORES)) if _core_ids is None else _core_ids

    def w_pcn(col0, col1):
        # [C, DPC] -> SBUF layout [p=128, c=CH, n=DPC], contiguous
        w = W_qkv_b[:, col0:col1].reshape(CH, 128, DPC)
        return np.ascontiguousarray(w.transpose(1, 0, 2))

    in_maps = []
    for core in range(len(core_ids)):
        s = slice(core * DPC, (core + 1) * DPC)
        in_maps.append(
            {
                "xT": xT,
                "wq": w_pcn(0 * C + core * DPC, 0 * C + (core + 1) * DPC),
                "wk": w_pcn(1 * C + core * DPC, 1 * C + (core + 1) * DPC),
                "wv": w_pcn(2 * C + core * DPC, 2 * C + (core + 1) * DPC),
                # device computes qT = psq*SCALE + bias, so pre-scale the q bias
                "bqkv": np.ascontiguousarray(
                    np.stack(
                        [
                            b_qkv[0 * C + core * DPC : 0 * C + (core + 1) * DPC] * np.float32(SCALE),
                            b_qkv[1 * C + core * DPC : 1 * C + (core + 1) * DPC],
                            b_qkv[2 * C + core * DPC : 2 * C + (core + 1) * DPC],
                        ],
                        axis=1,
                    )
                ),
                "wp": np.ascontiguousarray(W_proj_b[s, :]),
                "esel": esel_np,
            }
        )

    nc = _get_nc()
    res = run_bass_kernel_spmd(nc, in_maps, core_ids, trace=_trace)
    LAST_RESULT = res

    acc = np.zeros((NT, C), dtype=np.float64)
    for r in res.results:
        acc += r["out"].astype(np.float64)
    acc += b_proj.astype(np.float64)
    return acc.reshape(B, T, C).astype(np.float32)


# revision 42
# speedup vs baseline: 1.3123x; 1.0838x over previous
"""Causal self-attention layer (B=4, T=2048, C=1024, H=16) on 8 TRN2 NeuronCores.

Sharding: Megatron-style tensor parallel over heads — 2 heads per core.
Each core computes q/k/v projections for its 2 heads, causal flash-style
attention with ones-columns on V to accumulate softmax denominators, and a
partial output projection against its 128-row slice of W_proj. The host sums
the 8 partial projections and adds b_proj.

All matmul operands are bfloat16 (pre-cast on host for x/weights; on-chip
activations write bf16 directly out of PSUM). fp32r matmuls run in
fp32_mode=HIGH which power-throttles the PE to 50% for most of the kernel;
bf16 keeps the PE mostly at full rate, halves the x/out DMA bytes, and
doubles DVE throughput on the mask multiplies.

Schedule: the q/k/v projection tiles for batch b+1 are interleaved into the
attention stream of batch b, so the PE never runs a long unbroken stream of
dense projection matmuls (which exhausts the power budget and triggers the
50%-duty throttle), and no phase boundary stalls the PE. V tiles are
transposed once per batch ([128,128] both-heads transpose) with two ones
columns (at free offsets 64 and 129) so each head's 65-wide PV lhsT slice
puts y in psum rows 0-63 and the softmax denominator in row 64. The V
transposes for batch b+1 are emitted in the denominator-reciprocal latency
gap of batch b. PSUM: 4 pools x 2 bufs x 2KB = exactly 8 banks.
"""
import sys

sys.path.insert(0, "/opt/trn_rl_repo")

import numpy as np
import ml_dtypes

import concourse.bass as bass  # noqa: F401
from concourse import bacc
import concourse.mybir as mybir
import concourse.tile as tile
from concourse.bass_utils import run_bass_kernel_spmd
from concourse.masks import make_identity

B, T, C = 4, 2048, 1024
H, DH = 16, 64
N_CORES = 8
HPC = H // N_CORES          # heads per core = 2
DPC = HPC * DH              # head-dims per core = 128
NT = B * T                  # 8192 tokens
CH = C // 128               # 8 contraction chunks
QB = 512                    # q-block width (moving dim)
KT = 128                    # k-tile width (PE partition dim)
CHUNK = 8                   # k-tiles per S/PV emission chunk
SCALE = 1.0 / 8.0           # 1/sqrt(DH)
TPB = T // QB               # qkv token tiles per batch = 4

F32 = mybir.dt.float32
BF16 = mybir.dt.bfloat16
AF = mybir.ActivationFunctionType
BF16_NP = ml_dtypes.bfloat16

_CACHED_NC = None
LAST_RESULT = None


def _build():
    nc = bacc.Bacc(None)

    xT = nc.dram_tensor("xT", [C, NT], BF16, kind="ExternalInput")
    # qkv weights pre-arranged on host to the SBUF layout [p, c, n]
    wq = nc.dram_tensor("wq", [128, CH, DPC], BF16, kind="ExternalInput")
    wk = nc.dram_tensor("wk", [128, CH, DPC], BF16, kind="ExternalInput")
    wv = nc.dram_tensor("wv", [128, CH, DPC], BF16, kind="ExternalInput")
    bqkv = nc.dram_tensor("bqkv", [DPC, 3], F32, kind="ExternalInput")
    wp = nc.dram_tensor("wp", [DPC, C], BF16, kind="ExternalInput")
    # head selector: esel[0,:64]=1, esel[1,64:]=1 — broadcasts a [2,512]
    # reciprocal pair into a [128,512] per-dim normalizer via a K=2 matmul
    esel_in = nc.dram_tensor("esel", [2, 128], BF16, kind="ExternalInput")
    out = nc.dram_tensor("out", [NT, C], BF16, kind="ExternalOutput")

    with tile.TileContext(nc) as tc:
        with (
            tc.tile_pool(name="const", bufs=1) as const,
            tc.tile_pool(name="res", bufs=1) as res,
        ):
            # --- constants (built in f32, cast to bf16 once) ---
            ident = const.tile([128, 128], BF16, tag="ident")
            # sliding causal mask: wmask[k, u] = 1 iff k <= u - 512; a crossing
            # tile r multiplies by wmask[:, 512-128r : 1024-128r]
            wmask = const.tile([128, 1024], BF16, tag="wmask")
            ones_col = const.tile([128, 1], BF16, tag="ones_col")
            esel = const.tile([2, 128], BF16, tag="esel")
            with tc.tile_pool(name="cstage", bufs=1) as cstage:
                ident_s = cstage.tile([128, 128], F32, tag="ident_s")
                make_identity(nc, ident_s[:])
                nc.vector.tensor_copy(ident[:], ident_s[:])

                wmask_s = cstage.tile([128, 1024], F32, tag="wmask_s")
                nc.gpsimd.memset(wmask_s[:], 0.0)
                nc.gpsimd.affine_select(
                    out=wmask_s[:],
                    in_=wmask_s[:],
                    compare_op=mybir.AluOpType.is_gt,
                    fill=1.0,
                    base=512,
                    # keep 0 where (512 + k - u) > 0, fill 1 where k <= u - 512
                    pattern=[[-1, 1024]],
                    channel_multiplier=1,
                )
                nc.vector.tensor_copy(wmask[:], wmask_s[:])

                ones_s = cstage.tile([128, 1], F32, tag="ones_s")
                nc.gpsimd.memset(ones_s[:], 1.0)
                nc.vector.tensor_copy(ones_col[:], ones_s[:])

            bqkv_t = const.tile([DPC, 3], F32, tag="bqkv")
            bq_t, bk_t, bv_t = bqkv_t[:, 0:1], bqkv_t[:, 1:2], bqkv_t[:, 2:3]

            # weights -> SBUF directly in bf16 (cast on host)
            wq_r = const.tile([128, CH, DPC], BF16, tag="wq_r")
            wk_r = const.tile([128, CH, DPC], BF16, tag="wk_r")
            wv_r = const.tile([128, CH, DPC], BF16, tag="wv_r")
            wp_r = const.tile([DPC, C], BF16, tag="wp_r")

            # --- residents ---
            qT = res.tile([DPC, NT], BF16, tag="qT")
            kT = res.tile([DPC, NT], BF16, tag="kT")
            vT = res.tile([DPC, NT], BF16, tag="vT")
            yT = res.tile([DPC, NT], BF16, tag="yT")

            xT_re = xT.rearrange("(c p) t -> p c t", p=128)
            n_ktiles = T // KT  # 16

            with (
                tc.tile_pool(name="xpool", bufs=3) as xpool,
                tc.tile_pool(name="vpool", bufs=34) as vpool,
                tc.tile_pool(name="epool", bufs=CHUNK + 3) as epool,
                tc.tile_pool(name="dpool", bufs=2) as dpool,
                tc.tile_pool(name="opool", bufs=8) as opool,
                tc.tile_pool(name="q_psum", bufs=2, space="PSUM") as q_psum,
                tc.tile_pool(name="s_psum", bufs=3, space="PSUM") as s_psum,
                tc.tile_pool(name="y_psum", bufs=1, space="PSUM") as y_psum,
                tc.tile_pool(name="p_psum", bufs=2, space="PSUM") as p_psum,
            ):
                xs_tiles = {}

                def dma_x(tt):
                    if tt >= NT // QB or tt in xs_tiles:
                        return
                    xs = xpool.tile([128, CH, QB], BF16, tag="xs", name=f"xs{tt}")
                    if tt == 0:
                        # split the first tile across 4 DMA queues — it gates
                        # the very first matmul (a single queue moves ~110GB/s)
                        for c0 in range(0, CH, 2):
                            nc.sync.dma_start(
                                xs[:, c0 : c0 + 2, :],
                                xT_re[:, c0 : c0 + 2, tt * QB : (tt + 1) * QB],
                            )
                    else:
                        nc.sync.dma_start(
                            xs[:], xT_re[:, :, tt * QB : (tt + 1) * QB]
                        )
                    xs_tiles[tt] = xs

                def qkv_tile(tt):
                    """Project one 512-token tile into qT/kT/vT; prefetch x."""
                    dma_x(tt + 1)
                    xs = xs_tiles.pop(tt)
                    ts_ = slice(tt * QB, (tt + 1) * QB)
                    psq = q_psum.tile([128, QB], F32, tag="qkv", name=f"psq{tt}")
                    psk = q_psum.tile([128, QB], F32, tag="qkv", name=f"psk{tt}")
                    psv = q_psum.tile([128, QB], F32, tag="qkv", name=f"psv{tt}")
                    for ps, w_r in ((psq, wq_r), (psk, wk_r), (psv, wv_r)):
                        for c in range(CH):
                            nc.tensor.matmul(
                                ps[:], w_r[:, c, :], xs[:, c, :],
                                start=(c == 0), stop=(c == CH - 1),
                            )
                    # copy out of PSUM (+bias; q also scaled by 1/sqrt(dh))
                    nc.scalar.activation(qT[:, ts_], psq[:], AF.Identity, bias=bq_t[:], scale=SCALE)
                    nc.vector.tensor_scalar_add(kT[:, ts_], psk[:], bk_t[:])
                    nc.vector.tensor_scalar_add(vT[:, ts_], psv[:], bv_t[:])

                # per-batch state
                vts_all = {}   # b -> list of 16 [128, 130] tiles
                den_all = {}   # b -> (denw, den)

                def vts_half(b, half):
                    """Transpose 8 V token-tiles (both heads at once).

                    v tile layout [128 tok, 130]: cols 0-63 head0 dims, col 64
                    ones, cols 65-128 head1 dims, col 129 ones. Head hl's PV
                    lhsT is v[:, 65*hl : 65*hl+65] -> psum rows 0-63 = y,
                    row 64 = denominator.
                    """
                    cb = b * T
                    vts = vts_all.setdefault(b, [None] * n_ktiles)
                    for kt in range(half * 8, half * 8 + 8):
                        pt = s_psum.tile([128, 512], BF16, tag="s", name=f"pt{b}_{kt}")
                        nc.tensor.transpose(
                            pt[:, :128],
                            vT[:, cb + kt * KT : cb + (kt + 1) * KT],
                            ident[:],
                        )
                        v = vpool.tile([128, 130], BF16, tag="v", name=f"v{b}_{kt}")
                        nc.vector.tensor_copy(v[:, 0:64], pt[:, 0:64])
                        nc.scalar.copy(v[:, 65:129], pt[:, 64:128])
                        nc.vector.tensor_copy(v[:, 64:65], ones_col[:])
                        nc.vector.tensor_copy(v[:, 129:130], ones_col[:])
                        vts[kt] = v

                def sp_unit(b, hl, qb):
                    """Scores + exp + P@V for one (head, q-block)."""
                    cb = b * T
                    rb = hl * DH
                    vts = vts_all[b]
                    qs = slice(cb + qb * QB, cb + (qb + 1) * QB)
                    py = y_psum.tile([128, QB], F32, tag="py", name=f"py{b}_{hl}_{qb}")
                    nkt = (qb + 1) * (QB // KT)
                    # diagonal tile r's first 128*r q-columns are fully masked:
                    # narrow S/exp/mask/PV to [u0:] — no extra instructions
                    u0s = {
                        kt: max(kt - qb * (QB // KT), 0) * KT for kt in range(nkt)
                    }
                    for k0 in range(0, nkt, CHUNK):
                        kts = range(k0, min(k0 + CHUNK, nkt))
                        exs = {}
                        # scores + exp for this chunk
                        for kt in kts:
                            u0 = u0s[kt]
                            ps = s_psum.tile([128, QB], F32, tag="s", name=f"ps{kt}")
                            nc.tensor.matmul(
                                ps[:, u0:],
                                kT[rb : rb + DH, cb + kt * KT : cb + (kt + 1) * KT],
                                qT[rb : rb + DH, cb + qb * QB + u0 : cb + (qb + 1) * QB],
                                start=True,
                                stop=True,
                            )
                            ex = epool.tile([128, QB], BF16, tag="ex", name=f"ex{kt}")
                            nc.scalar.activation(ex[:, u0:], ps[:, u0:], AF.Exp)
                            r = kt - qb * (QB // KT)
                            if r >= 0:
                                # diagonal-crossing tile: zero out k > q
                                nc.vector.tensor_mul(
                                    ex[:, u0:], ex[:, u0:],
                                    wmask[:, 512 : 1024 - u0],
                                )
                            exs[kt] = ex
                        # grouped P@V accumulation for this chunk
                        for kt in kts:
                            u0 = u0s[kt]
                            nc.tensor.matmul(
                                py[: DH + 1, u0:],
                                vts[kt][:, 65 * hl : 65 * hl + 65],
                                exs[kt][:, u0:],
                                start=(kt == 0),
                                stop=(kt == nkt - 1),
                            )
                    # stash unnormalized y; scatter the denominator row into
                    # this q-block's [2, QB] pair tile (DMA may write any
                    # partition; compute engines may not)
                    denw, dens = den_all[b]
                    p = 2 * qb + hl
                    nc.vector.tensor_copy(
                        denw[:, p * QB : (p + 1) * QB], py[DH : DH + 1, :]
                    )
                    nc.vector.tensor_copy(yT[rb : rb + DH, qs], py[:DH, :])
                    nc.sync.dma_start(
                        dens[qb][hl : hl + 1, :], denw[:, p * QB : (p + 1) * QB]
                    )

                def norm_qb(b, qb):
                    """Reciprocal + normalize for one q-block (both heads)."""
                    _denw, dens = den_all[b]
                    cb = b * T
                    qs = slice(cb + qb * QB, cb + (qb + 1) * QB)
                    recf = dpool.tile([2, QB], F32, tag="recf", name=f"recf{b}_{qb}")
                    nc.vector.reciprocal_approx_fast(recf[:], dens[qb][:])
                    rec = dpool.tile([2, QB], BF16, tag="rec", name=f"rec{b}_{qb}")
                    if b == B - 1 and qb >= 2:
                        # endgame: no exps left, ACT is idle while DVE drains
                        nc.scalar.copy(rec[:], recf[:])
                    else:
                        nc.vector.tensor_copy(rec[:], recf[:])
                    pb = p_psum.tile([128, 512], F32, tag="p", name=f"pb{b}_{qb}")
                    nc.tensor.matmul(
                        pb[:, :QB], esel[:], rec[:],
                        start=True, stop=True,
                    )
                    nc.vector.tensor_mul(yT[:, qs], yT[:, qs], pb[:, :QB])

                def proj_quarter(b, i, deep=False):
                    """Output projection for 4 of the batch's 16 token tiles.

                    deep=True borrows the idle y_psum bank for a 3-deep psum
                    rotation (only safe when no sp unit is accumulating py).
                    """
                    act_heavy = b == B - 1 and i >= 2
                    cb = b * T
                    n = 0
                    for tt in range(i * 4, i * 4 + 4):
                        trow = cb + tt * 128
                        for half in range(2):
                            pool = y_psum if (deep and n % 3 == 2) else p_psum
                            tag = "py" if (deep and n % 3 == 2) else "p"
                            pp = pool.tile([128, 512], F32, tag=tag, name=f"pp{b}_{tt}_{half}")
                            n += 1
                            nc.tensor.matmul(
                                pp[:],
                                yT[:, trow : trow + 128],
                                wp_r[:, half * 512 : (half + 1) * 512],
                                start=True,
                                stop=True,
                            )
                            os_ = opool.tile([128, 512], BF16, tag="os", name=f"os{b}_{tt}_{half}")
                            # copy engines 2:1 DVE:ACT — ACT carries the exps;
                            # flipped in the endgame where ACT is idle
                            on_act = (tt * 2 + half) % 3 == 2
                            if act_heavy:
                                on_act = not on_act
                            if on_act:
                                nc.scalar.copy(os_[:], pp[:])
                            else:
                                nc.vector.tensor_copy(os_[:], pp[:])
                            nc.sync.dma_start(
                                out[trow : trow + 128, half * 512 : (half + 1) * 512],
                                os_[:],
                            )

                # ================= emission schedule =================
                # Per batch b: 8 sp units (hl pairs per q-block). After each
                # pair, that q-block is normalized (per-qb reciprocal via a
                # K=2 selector matmul). ALL proj quarters are carried into the
                # next batch's attention stream as fillers, woven 1:1 with the
                # next batch's qkv tiles, so full-power matmul runs never
                # cluster and the serial tail is one quarter. The V transposes
                # for batch b+1 form the (low-power) end block.
                # DMA issue order: the first x tile and wq gate the first
                # matmul, so they go first (issues serialize at ~650ns each).
                dma_x(0)
                nc.sync.dma_start(wq_r[:], wq[:])
                nc.sync.dma_start(wk_r[:], wk[:])
                nc.sync.dma_start(wv_r[:], wv[:])
                nc.sync.dma_start(bqkv_t[:], bqkv[:])
                nc.sync.dma_start(wp_r[:], wp[:])
                nc.sync.dma_start(esel[:], esel_in[:])
                # qkv for batch 0, with its V transposes woven in (each vts
                # half only needs the first two / all four tiles' vT)
                qkv_tile(0)
                qkv_tile(1)
                vts_half(0, 0)
                qkv_tile(2)
                qkv_tile(3)
                vts_half(0, 1)
                for b in range(B):
                    denw = dpool.tile([1, 8 * QB], F32, tag="denw", name=f"denw{b}")
                    dens = [
                        dpool.tile([2, QB], F32, tag="den", name=f"den{b}_{qb}")
                        for qb in range(T // QB)
                    ]
                    den_all[b] = (denw, dens)
                    # fillers[i] runs after sp unit i: previous batch's proj
                    # quarters and next batch's qkv tiles, alternating
                    fillers = {}
                    if b == 0:
                        for j, pos in enumerate((2, 3, 5, 6)):
                            fillers[pos] = lambda tt=TPB + j: qkv_tile(tt)
                    elif b < B - 1:
                        fillers = {
                            0: lambda: proj_quarter(b - 1, 0),
                            1: lambda: qkv_tile((b + 1) * TPB + 0),
                            2: lambda: proj_quarter(b - 1, 1),
                            3: lambda: qkv_tile((b + 1) * TPB + 1),
                            4: lambda: proj_quarter(b - 1, 2),
                            5: lambda: qkv_tile((b + 1) * TPB + 2),
                            6: lambda: proj_quarter(b - 1, 3),
                            7: lambda: qkv_tile((b + 1) * TPB + 3),
                        }
                    else:
                        # last batch: weave in b-1's quarters and our own
                        # earlier-normalized quarters
                        fillers = {
                            0: lambda: proj_quarter(b - 1, 0),
                            2: lambda: proj_quarter(b - 1, 1),
                            3: lambda: proj_quarter(b, 0),
                            4: lambda: proj_quarter(b - 1, 2),
                            5: lambda: proj_quarter(b, 1),
                            6: lambda: proj_quarter(b - 1, 3),
                            7: lambda: proj_quarter(b, 2),
                        }
                    sps = [(hl, qb) for qb in range(T // QB) for hl in range(HPC)]
                    for i, (hl, qb) in enumerate(sps):
                        sp_unit(b, hl, qb)
                        # filler first: its matmuls cover the reciprocal
                        # chain's cross-engine latency before the pb matmul
                        f = fillers.get(i)
                        if f is not None:
                            f()
                        if hl == HPC - 1:
                            norm_qb(b, qb)
                    if b + 1 < B:
                        vts_half(b + 1, 0)
                        vts_half(b + 1, 1)
                        vts_all.pop(b, None)
                    else:
                        proj_quarter(b, 3, deep=True)

    nc.compile()
    return nc


def _get_nc():
    global _CACHED_NC
    if _CACHED_NC is None:
        _CACHED_NC = _build()
    return _CACHED_NC


def kernel(x, W_qkv, b_qkv, W_proj, b_proj, _trace=False, _core_ids=None):
    global LAST_RESULT
    x = np.asarray(x, dtype=np.float32)
    W_qkv = np.asarray(W_qkv, dtype=np.float32)
    b_qkv = np.asarray(b_qkv, dtype=np.float32)
    W_proj = np.asarray(W_proj, dtype=np.float32)
    b_proj = np.asarray(b_proj, dtype=np.float32)

    xT = np.ascontiguousarray(x.reshape(NT, C).T).astype(BF16_NP)
    W_qkv_b = W_qkv.astype(BF16_NP)
    W_proj_b = W_proj.astype(BF16_NP)
    esel_np = np.zeros((2, 128), dtype=BF16_NP)
    esel_np[0, :DH] = 1.0
    esel_np[1, DH:] = 1.0
    core_ids = list(range(N_CORES)) if _core_ids is None else _core_ids

    def w_pcn(col0, col1):
        # [C, DPC] -> SBUF layout [p=128, c=CH, n=DPC], contiguous
        w = W_qkv_b[:, col0:col1].reshape(CH, 128, DPC)
        return np.ascontiguousarray(w.transpose(1, 0, 2))

    in_maps = []
    for core in range(len(core_ids)):
        s = slice(core * DPC, (core + 1) * DPC)
        in_maps.append(
            {
                "xT": xT,
                "wq": w_pcn(0 * C + core * DPC, 0 * C + (core + 1) * DPC),
                "wk": w_pcn(1 * C + core * DPC, 1 * C + (core + 1) * DPC),
                "wv": w_pcn(2 * C + core * DPC, 2 * C + (core + 1) * DPC),
                # device computes qT = psq*SCALE + bias, so pre-scale the q bias
                "bqkv": np.ascontiguousarray(
                    np.stack(
                        [
                            b_qkv[0 * C + core * DPC : 0 * C + (core + 1) * DPC] * np.float32(SCALE),
                            b_qkv[1 * C + core * DPC : 1 * C + (core + 1) * DPC],
                            b_qkv[2 * C + core * DPC : 2 * C + (core + 1) * DPC],
                        ],
                        axis=1,
                    )
                ),
                "wp": np.ascontiguousarray(W_proj_b[s, :]),
                "esel": esel_np,
            }
        )

    nc = _get_nc()
    res = run_bass_kernel_spmd(nc, in_maps, core_ids, trace=_trace)
    LAST_RESULT = res

    acc = np.zeros((NT, C), dtype=np.float64)
    for r in res.results:
        acc += r["out"].astype(np.float64)
    acc += b_proj.astype(np.float64)
    return acc.reshape(B, T, C).astype(np.float32)
